# revision 5
# baseline (speedup 1.0000x reference)
"""Trainium2 Bass kernel for 2-layer GAT (nn_GAT_43765716746408) — v2.

Differences vs v1 baseline (5.15 ms):
  - No er dma_gather: per-edge er(dst) is selected on the tensor engine.
    Per 128-edge group: replicate the group's dst ids to all partitions
    with a ones-column matmul (PSUM), build the transposed one-hot
    ST[d, e] = (dst[e] == d) with one is_equal, then ere = ST^T @ er2
    (fp32 matmul, 2 streamed columns).
  - No 2-port DVE ops (tensor_scalar / copy / cast) concurrent with SWDGE
    descriptor generation: the DVE<->GpSimd shared SBUF port lock was
    serializing the edge phase. Hot-path element-wise work uses
    tensor_tensor / reduce (single-port classes) or the scalar engine.
  - Aggregation uses weight-scaled one-hots: Swh = S * w_h, and tbig rows
    carry a literal 1.0 column per head so the denominators fall out of
    the same matmuls (no strided w copies into msg).
  - leaky-relu on DVE as max(u, 0.2u) (two tensor_tensor), exp on ACT.
  - tbig is double-buffered across layers so layer-1 P1 can overlap
    layer-0's edge phase.
"""

import sys

sys.path.insert(0, "/opt/trn_rl_repo")

import numpy as np

import concourse.bass as bass
import concourse.tile as tile
from concourse import bacc, mybir
from concourse.bass_utils import run_bass_kernel_spmd
from concourse.masks import make_identity

F32 = mybir.dt.float32
F16 = mybir.dt.float16
I16 = mybir.dt.int16
I32 = mybir.dt.int32

N_CORES = 8
D = 128          # model dim
H = 2            # heads
HF = 256         # H * F
ROW16 = 384      # fp16 elements per TBIG row (768B pitch)
NEG_SLOPE = 0.2

# tbig row layout (fp16 elems):
#   [0:128]   f0 (head-0 features)
#   [128]     1.0           (denominator column for head 0)
#   [129:257] f1
#   [257]     1.0           (denominator column for head 1)
#   [258:262] el0, el1 as two f32 (bitcast)
#   [262:384] pad (never read)


class Cfg:
    def __init__(self, n_nodes, n_edges, n_layers=2):
        assert n_nodes % N_CORES == 0
        self.n = n_nodes
        self.e = n_edges
        self.layers = n_layers
        self.nloc = n_nodes // N_CORES
        self.t = -(-self.nloc // 128)          # dst tiles per core
        self.nloc_pad = self.t * 128
        self.w_last = self.nloc - 128 * (self.t - 1)
        self.split = n_nodes // 2              # lo/hi table split (int16 range)
        assert self.split < 32768 and (n_nodes - self.split) < 32768


FULL = Cfg(50000, 800000)


# ----------------------------------------------------------------------------
# Host-side edge preprocessing
# ----------------------------------------------------------------------------

def prep_edges(cfg, src, dst):
    """Bucket edges per core by (dst_tile, src_half); pad to shared sizes."""
    C, T = N_CORES, cfg.t
    counts = np.zeros((C, T, 2), dtype=np.int64)
    per_core = []
    core_of = dst // cfg.nloc
    for c in range(C):
        sel = core_of == c
        es, ed = src[sel].astype(np.int64), dst[sel].astype(np.int64)
        dloc = ed - c * cfg.nloc
        t = dloc // 128
        half = (es >= cfg.split).astype(np.int64)
        order = np.lexsort((es, half, t))
        es, dloc, t, half = es[order], dloc[order], t[order], half[order]
        np.add.at(counts[c], (t, half), 1)
        per_core.append((es, dloc, t, half))

    gmax_th = counts.max(axis=0)                       # (T, 2)
    G = np.maximum(1, -(-gmax_th // 128))              # groups per (t, half)
    base = np.zeros((T, 2), dtype=np.int64)
    acc = 0
    for t in range(T):
        for h in range(2):
            base[t, h] = acc
            acc += G[t, h]
    gtot = acc
    nslot = gtot * 128

    src_idx = np.zeros((C, nslot), dtype=np.int16)
    dst_reb = np.full((C, nslot), -1.0, dtype=np.float16)
    for c in range(C):
        es, dloc, t, half = per_core[c]
        bucket_id = t * 2 + half
        n = len(es)
        starts = np.searchsorted(bucket_id, np.arange(T * 2), side="left")
        pos_in_bucket = np.arange(n) - starts[bucket_id]
        slot = base[t, half] * 128 + pos_in_bucket
        src_idx[c, slot] = (es - np.where(half == 1, cfg.split, 0)).astype(np.int16)
        dst_reb[c, slot] = (dloc - t * 128).astype(np.float32)

    def wrap16(a):
        w = a.reshape(-1, 16).T.copy()                 # (16, nslot/16)
        return np.tile(w, (8, 1))                      # (128, nslot/16)

    src_w = np.stack([wrap16(src_idx[c]) for c in range(C)])
    # dst one-hot source, partition-major per group: dst_w[c][p, g] = dst of
    # slot (p, g)
    dst_w = np.stack([dst_reb[c].reshape(-1, 128).T.copy() for c in range(C)])
    # slot-major rows per tile for the ST replication matmul:
    # dstrow[c][t, g*128 + p] = dst of slot (p, g) of tile t (or -1 pad)
    gt_n = G[:, 0] + G[:, 1]
    gmax = int(gt_n.max())
    dstrow = np.full((C, T, gmax * 128), -1.0, dtype=np.float16)
    for c in range(C):
        flat = dst_reb[c]
        for t in range(T):
            g0 = int(base[t, 0])
            n_in_tile = int(gt_n[t]) * 128
            dstrow[c, t, :n_in_tile] = flat[g0 * 128 : g0 * 128 + n_in_tile]

    geom = {"G": G, "base": base, "gtot": gtot, "gmax": gmax}
    return geom, src_w, dst_w, dstrow


def prep_weights(cfg, Ws, als, ars, bs):
    """Combined matmul weights Wc = [W | wl | wr] and packed bias rows."""
    L = cfg.layers
    wc = np.zeros((L, D, HF + 4), dtype=np.float16)
    bp = np.zeros((L, 1, 384), dtype=np.float16)
    for l in range(L):
        W = np.asarray(Ws[l], dtype=np.float32)            # (D, H*F)
        Wh = W.reshape(D, H, D)
        wl = np.einsum("khf,hf->kh", Wh, np.asarray(als[l], np.float32))
        wr = np.einsum("khf,hf->kh", Wh, np.asarray(ars[l], np.float32))
        wc[l, :, :HF] = W.astype(np.float16)
        wc[l, :, HF : HF + 2] = wl.astype(np.float16)
        wc[l, :, HF + 2 : HF + 4] = wr.astype(np.float16)
        b = np.asarray(bs[l], np.float32)
        bp[l, 0, 0:128] = b[0].astype(np.float16)
        bp[l, 0, 128:256] = b[1].astype(np.float16)
        bp[l, 0, 256:384] = (0.5 * (b[0] + b[1])).astype(np.float16)
    return wc, bp


# ----------------------------------------------------------------------------
# Device kernel
# ----------------------------------------------------------------------------

def build(cfg, geom):
    C, T, L = N_CORES, cfg.t, cfg.layers
    G, base, gtot, gmax = geom["G"], geom["base"], geom["gtot"], geom["gmax"]
    nslot = gtot * 128
    NLO = cfg.split

    nc = bacc.Bacc("TRN2", target_bir_lowering=False, debug=False,
                   enable_asserts=False, num_devices=C, num_swdge_queues=2)

    # I/O
    xTb = nc.dram_tensor("xTb", [C, D, cfg.nloc], F16, kind="ExternalInput")
    wc_d = nc.dram_tensor("wc", [L, D, HF + 4], F16, kind="ExternalInput")
    bp_d = nc.dram_tensor("bp", [L, 1, 384], F16, kind="ExternalInput")
    src_d = nc.dram_tensor("srcw", [D, nslot // 16], I16, kind="ExternalInput")
    dst_d = nc.dram_tensor("dstw", [D, gtot], F16, kind="ExternalInput")
    dstrow_d = nc.dram_tensor("dstrow", [T, gmax * 128], F16, kind="ExternalInput")
    oh_d = nc.dram_tensor("onehot", [D, 8], F32, kind="ExternalInput")
    out_d = nc.dram_tensor("out", [cfg.nloc_pad, HF], F32, kind="ExternalOutput")

    # internal DRAM (tbig double-buffered across layers)
    tbig = [nc.dram_tensor(f"tbig{l}", [cfg.n, ROW16], F16) for l in range(L)]
    THALF = (T + 1) // 2
    ch0 = min(THALF * 128, cfg.nloc)
    ch1 = cfg.nloc - ch0
    hT_own0 = nc.dram_tensor("hT_own0", [D, ch0], F16)
    hT_own1 = nc.dram_tensor("hT_own1", [D, ch1], F16)
    hT_all0 = nc.dram_tensor("hT_all0", [C, D, ch0], F16, addr_space="Shared")
    hT_all1 = nc.dram_tensor("hT_all1", [C, D, ch1], F16, addr_space="Shared")

    with tile.TileContext(nc) as tc:
        with (
            tc.tile_pool(name="const", bufs=1) as cpool,
            tc.tile_pool(name="work", bufs=2) as pool,
            tc.tile_pool(name="gtp", bufs=3) as gtp,
            tc.tile_pool(name="ppA", bufs=3, space="PSUM") as ppA,
            tc.tile_pool(name="ppB", bufs=1, space="PSUM") as ppB,
        ):
            # ---- constants ----
            src_sb = cpool.tile([D, nslot // 16], I16, tag="src_sb")
            nc.sync.dma_start(out=src_sb[:], in_=src_d[:])
            dst_sb = cpool.tile([D, gtot], F16, tag="dst_sb")
            nc.sync.dma_start(out=dst_sb[:], in_=dst_d[:])

            oh_sb = cpool.tile([D, 8], F32, tag="oh_sb")
            nc.sync.dma_start(out=oh_sb[:], in_=oh_d[:])
            wc_sb = cpool.tile([D, L * (HF + 4)], F16, tag="wc_sb")
            bp_sb = cpool.tile([1, L * 384], F16, tag="bp_sb")
            for l in range(L):
                nc.sync.dma_start(
                    out=wc_sb[:, l * (HF + 4) : (l + 1) * (HF + 4)], in_=wc_d[l]
                )
                nc.sync.dma_start(
                    out=bp_sb[:, l * 384 : (l + 1) * 384], in_=bp_d[l]
                )

            it16 = cpool.tile([D, D], I16, tag="it16")
            nc.gpsimd.iota(it16[:], pattern=[[1, D]], base=0, channel_multiplier=0)
            iota_f = cpool.tile([D, D], F16, tag="iota_f")
            nc.vector.tensor_copy(iota_f[:], it16[:])

            ip32 = cpool.tile([D, 1], I32, tag="ip32")
            nc.gpsimd.iota(ip32[:], pattern=[[1, 1]], base=0, channel_multiplier=1)
            iota_p16 = cpool.tile([D, 1], F16, tag="iota_p16")
            nc.vector.tensor_copy(iota_p16[:], ip32[:])

            ones_row = cpool.tile([1, D], F16, tag="ones_row")
            nc.vector.memset(ones_row[:], 1.0)
            ident = cpool.tile([D, D], F16, tag="ident")
            make_identity(nc, ident[:])

            epsc = cpool.tile([D, 1], F32, tag="epsc")
            nc.vector.memset(epsc[:], 1e-30)
            halfc = cpool.tile([D, 1], F32, tag="halfc")
            nc.vector.memset(halfc[:], 0.5)
            slopec = cpool.tile([D, 1], F32, tag="slopec")
            nc.vector.memset(slopec[:], NEG_SLOPE)
            zc32 = cpool.tile([D, 1], F32, tag="zc32")
            nc.vector.memset(zc32[:], 0.0)
            zc16 = cpool.tile([D, 1], F16, tag="zc16")
            nc.vector.memset(zc16[:], 0.0)

            # layer-indexed staging (layers overlap: tbig double-buffered)
            er_stage = cpool.tile([D, L, T, 2, 8], F32, tag="er_stage")
            hT_stage = cpool.tile([D, cfg.nloc_pad], F16, tag="hT_stage")
            brep_t = cpool.tile([D, L, 384], F32, tag="brep")

            for l in range(L):
                brep = brep_t[:, l]
                # ---- bias broadcast to all partitions (PE trick, 3 chunks
                # through the shared [D, 128] psum tag) ----
                for k in range(3):
                    bps = ppA.tile([D, HF + 4], F32, tag="p1")
                    nc.tensor.matmul(
                        bps[:, 0:128], lhsT=ones_row[:],
                        rhs=bp_sb[:, l * 384 + k * 128 : l * 384 + (k + 1) * 128],
                        start=True, stop=True,
                    )
                    nc.vector.tensor_tensor(
                        out=brep[:, k * 128 : (k + 1) * 128], in0=bps[:, 0:128],
                        in1=zc32[:, 0:1].to_broadcast([D, D]),
                        op=mybir.AluOpType.add,
                    )
                nc.vector.memset(er_stage[:, l], 0.0)

                # ---- P1: feat/el/er table build (replicated over all nodes) ----
                wcl = wc_sb[:, l * (HF + 4) : l * (HF + 4) + HF + 4]
                tb = tbig[l]
                with nc.named_scope(f"p1_l{l}"):
                    for cb in range(C):
                        t = 0
                        while t < T:
                            # pairs of full tiles; tail tile alone; don't
                            # straddle the hT half boundary
                            nt = 2 if (t + 2 <= T and t + 1 != T - 1) else 1
                            if t + nt > T or (t < THALF <= t + 1):
                                nt = 1
                            w2 = 0
                            ws = []
                            for j in range(nt):
                                wj = cfg.w_last if t + j == T - 1 else 128
                                ws.append(wj)
                                w2 += wj
                            xt = pool.tile([D, 256], F16, tag="xt")
                            if l == 0:
                                src_ap = xTb[cb, :, t * 128 : t * 128 + w2]
                            elif t < THALF:
                                src_ap = hT_all0[cb, :, t * 128 : t * 128 + w2]
                            else:
                                c0 = t * 128 - ch0
                                src_ap = hT_all1[cb, :, c0 : c0 + w2]
                            nc.sync.dma_start(out=xt[:, :w2], in_=src_ap)
                            stage = pool.tile([D, 2, 384], F16, tag="stage")
                            for j in range(nt):
                                wj = ws[j]
                                ps1 = ppA.tile([D, HF + 4], F32, tag="p1")
                                nc.tensor.matmul(
                                    ps1[:wj, :], lhsT=xt[:, j * 128 : j * 128 + wj],
                                    rhs=wcl, start=True, stop=True,
                                )
                                # f0 | f1 at cols 0:128 / 129:257 (stride 129)
                                sv = stage[:, j, 0:258].rearrange(
                                    "p (h v) -> p h v", h=2
                                )
                                nc.scalar.activation(
                                    sv[:wj, :, 0:128],
                                    ps1[:wj, 0:HF].rearrange(
                                        "p (h v) -> p h v", h=2
                                    ),
                                    mybir.ActivationFunctionType.Copy,
                                )
                                nc.scalar.activation(
                                    stage[:wj, j, 258:262].bitcast(F32),
                                    ps1[:wj, HF : HF + 2],
                                    mybir.ActivationFunctionType.Copy,
                                )
                                nc.vector.tensor_tensor(
                                    out=er_stage[:wj, l, t + j, :, cb],
                                    in0=ps1[:wj, HF + 2 : HF + 4],
                                    in1=zc32[:wj, 0:1].to_broadcast([wj, 2]),
                                    op=mybir.AluOpType.add,
                                )
                            nc.vector.memset(stage[:, :, 128:129], 1.0)
                            nc.vector.memset(stage[:, :, 257:258], 1.0)
                            n0 = cb * cfg.nloc + t * 128
                            if nt == 2:
                                tbv = tb[n0 : n0 + 256, :].rearrange(
                                    "(j p) v -> p j v", p=128
                                )
                                nc.sync.dma_start(
                                    out=tbv[:, :, 0:262],
                                    in_=stage[:, :, 0:262],
                                )
                            else:
                                nc.sync.dma_start(
                                    out=tb[n0 : n0 + ws[0], 0:262],
                                    in_=stage[: ws[0], 0, 0:262],
                                )
                            t += nt

                # ---- P2: edge phase ----
                with nc.named_scope(f"p2_l{l}"):
                    for t in range(T):
                        g_lo, g_hi = int(G[t, 0]), int(G[t, 1])
                        gt_n = g_lo + g_hi
                        goff = int(base[t, 0])

                        gt = gtp.tile([D, gmax, ROW16], F16, tag="gt")
                        nc.gpsimd.dma_gather(
                            out_ap=gt[:, 0:g_lo, :],
                            in_ap=tb[0:NLO, :],
                            idxs_ap=src_sb[:, goff * 8 : (goff + g_lo) * 8],
                            num_idxs=g_lo * 128,
                            num_idxs_reg=g_lo * 128,
                            elem_size=ROW16,
                            queue_num=0,
                            single_packet=False,
                        )
                        nc.gpsimd.dma_gather(
                            out_ap=gt[:, g_lo:gt_n, :],
                            in_ap=tb[NLO : cfg.n, :],
                            idxs_ap=src_sb[:, (goff + g_lo) * 8 : (goff + gt_n) * 8],
                            num_idxs=g_hi * 128,
                            num_idxs_reg=g_hi * 128,
                            elem_size=ROW16,
                            queue_num=1,
                            single_packet=False,
                        )

                        # er2[d, h] = own-core er of node t*128+d (fp32)
                        tmp8 = pool.tile([D, 2, 8], F32, tag="tmp8")
                        nc.vector.tensor_tensor(
                            out=tmp8[:],
                            in0=er_stage[:, l, t],
                            in1=oh_sb[:].unsqueeze(1).to_broadcast([D, 2, 8]),
                            op=mybir.AluOpType.mult,
                        )
                        er2f = pool.tile([D, 2], F32, tag="er2f")
                        nc.vector.reduce_sum(er2f[:], tmp8[:], axis=mybir.AxisListType.X)
                        er2 = pool.tile([D, 2], F16, tag="er2")
                        nc.vector.tensor_tensor(
                            out=er2[:], in0=er2f[:],
                            in1=zc32[:, 0:1].to_broadcast([D, 2]),
                            op=mybir.AluOpType.add,
                        )

                        # per-group: replicate dst row (PE), transposed one-hot
                        # ST (DVE is_equal), ere = ST^T @ er2 (PE, 2 cols)
                        drep = pool.tile([D, gmax * 128], F16, tag="drep")
                        nc.sync.dma_start(
                            out=drep[:, 0 : gt_n * 128],
                            in_=dstrow_d[t : t + 1, 0 : gt_n * 128].to_broadcast(
                                [D, gt_n * 128]
                            ),
                        )
                        ST = pool.tile([D, gmax, D], F16, tag="ST")
                        nc.vector.tensor_tensor(
                            out=ST[:, 0:gt_n],
                            in0=drep[:, 0 : gt_n * 128].rearrange(
                                "p (g e) -> p g e", e=D
                            ),
                            in1=iota_p16[:, 0:1].unsqueeze(1).to_broadcast(
                                [D, gt_n, D]
                            ),
                            op=mybir.AluOpType.is_equal,
                        )
                        ps_ere = ppB.tile([D, gmax, 2], F32, tag="ere")
                        for g in range(gt_n):
                            nc.tensor.matmul(
                                ps_ere[:, g, :],
                                lhsT=ST[:, g, :],
                                rhs=er2[:],
                                start=True, stop=True,
                            )

                        # u = el_src + er_dst ; w = exp(max(u, 0.2u))
                        elv = gt[:, 0:gt_n, 258:262].bitcast(F32)
                        u = pool.tile([D, gmax, 2], F32, tag="u")
                        nc.vector.tensor_tensor(
                            out=u[:, 0:gt_n], in0=ps_ere[:, 0:gt_n], in1=elv,
                            op=mybir.AluOpType.add,
                        )
                        lr = pool.tile([D, gmax, 2], F32, tag="lr")
                        nc.vector.tensor_tensor(
                            out=lr[:, 0:gt_n], in0=u[:, 0:gt_n],
                            in1=slopec[:, 0:1].unsqueeze(1).to_broadcast([D, gt_n, 2]),
                            op=mybir.AluOpType.mult,
                        )
                        nc.vector.tensor_tensor(
                            out=u[:, 0:gt_n], in0=u[:, 0:gt_n], in1=lr[:, 0:gt_n],
                            op=mybir.AluOpType.max,
                        )
                        w16 = pool.tile([D, gmax, 2], F16, tag="w16")
                        nc.scalar.activation(
                            w16[:, 0:gt_n], u[:, 0:gt_n],
                            mybir.ActivationFunctionType.Exp,
                        )

                        # one-hot S[e, d] and per-head scaled copies
                        S = pool.tile([D, gmax, D], F16, tag="S")
                        nc.vector.tensor_tensor(
                            out=S[:, 0:gt_n],
                            in0=dst_sb[:, goff : goff + gt_n].unsqueeze(2).to_broadcast(
                                [D, gt_n, D]
                            ),
                            in1=iota_f[:].unsqueeze(1).to_broadcast([D, gt_n, D]),
                            op=mybir.AluOpType.is_equal,
                        )
                        Sw0 = pool.tile([D, gmax, D], F16, tag="Sw0")
                        nc.vector.tensor_tensor(
                            out=Sw0[:, 0:gt_n],
                            in0=S[:, 0:gt_n],
                            in1=w16[:, 0:gt_n, 0:1].to_broadcast([D, gt_n, D]),
                            op=mybir.AluOpType.mult,
                        )
                        Sw1 = pool.tile([D, gmax, D], F16, tag="Sw1")
                        nc.vector.tensor_tensor(
                            out=Sw1[:, 0:gt_n],
                            in0=S[:, 0:gt_n],
                            in1=w16[:, 0:gt_n, 1:2].to_broadcast([D, gt_n, D]),
                            op=mybir.AluOpType.mult,
                        )

                        # aggregation: two sequential chains sharing one bank
                        # (start=True clears accumulate flags bank-wide, so the
                        # chains must not interleave)
                        ps_agg = ppA.tile([D, 258], F32, tag="agg")
                        ps_a0 = ps_agg[:, 0:129]
                        ps_a1 = ps_agg[:, 129:258]
                        for g in range(gt_n):
                            nc.tensor.matmul(
                                ps_a0,
                                lhsT=Sw0[:, g, :],
                                rhs=gt[:, g, 0:129],
                                start=(g == 0),
                                stop=(g == gt_n - 1),
                            )
                        for g in range(gt_n):
                            nc.tensor.matmul(
                                ps_a1,
                                lhsT=Sw1[:, g, :],
                                rhs=gt[:, g, 129:258],
                                start=(g == 0),
                                stop=(g == gt_n - 1),
                            )

                        # ---- finalize ----
                        rsb = pool.tile([D, 2], F32, tag="rsb")
                        nc.vector.tensor_tensor(
                            out=rsb[:, 0:1], in0=ps_agg[:, 128:129], in1=epsc[:],
                            op=mybir.AluOpType.max,
                        )
                        nc.vector.tensor_tensor(
                            out=rsb[:, 1:2], in0=ps_agg[:, 257:258], in1=epsc[:],
                            op=mybir.AluOpType.max,
                        )
                        nc.vector.reciprocal(rsb[:], rsb[:])
                        if l == 0:
                            rh = pool.tile([D, 2], F32, tag="rh")
                            nc.vector.tensor_tensor(
                                out=rh[:], in0=rsb[:],
                                in1=halfc[:, 0:1].to_broadcast([D, 2]),
                                op=mybir.AluOpType.mult,
                            )
                            t0 = pool.tile([D, D], F32, tag="t0")
                            nc.vector.tensor_tensor(
                                out=t0[:], in0=ps_agg[:, 0:128],
                                in1=rh[:, 0:1].to_broadcast([D, D]),
                                op=mybir.AluOpType.mult,
                            )
                            t1 = pool.tile([D, D], F32, tag="t1")
                            nc.vector.tensor_tensor(
                                out=t1[:], in0=ps_agg[:, 129:257],
                                in1=rh[:, 1:2].to_broadcast([D, D]),
                                op=mybir.AluOpType.mult,
                            )
                            nc.vector.tensor_tensor(
                                out=t0[:], in0=t0[:], in1=t1[:],
                                op=mybir.AluOpType.add,
                            )
                            ht16 = pool.tile([D, D], F16, tag="ht16")
                            nc.vector.tensor_tensor(
                                out=ht16[:], in0=t0[:], in1=brep[:, 256:384],
                                op=mybir.AluOpType.add,
                            )
                            pst = ppB.tile([D, D], F16, tag="pt")
                            nc.tensor.transpose(pst[:], ht16[:], ident[:])
                            nc.vector.tensor_tensor(
                                out=hT_stage[:, t * 128 : (t + 1) * 128],
                                in0=pst[:],
                                in1=zc16[:, 0:1].to_broadcast([D, D]),
                                op=mybir.AluOpType.add,
                            )
                            if t == THALF - 1:
                                nc.sync.dma_start(
                                    out=hT_own0[:], in_=hT_stage[:, 0:ch0]
                                )
                                with nc.named_scope("cc0"):
                                    nc.gpsimd.collective_compute(
                                        "AllGather",
                                        mybir.AluOpType.bypass,
                                        replica_groups=[list(range(C))],
                                        ins=[hT_own0[:]],
                                        outs=[hT_all0[:]],
                                    )
                        else:
                            osb = pool.tile([D, HF], F32, tag="osb")
                            nc.vector.tensor_tensor(
                                out=osb[:, 0:128], in0=ps_agg[:, 0:128],
                                in1=rsb[:, 0:1].to_broadcast([D, D]),
                                op=mybir.AluOpType.mult,
                            )
                            nc.vector.tensor_tensor(
                                out=osb[:, 128:256], in0=ps_agg[:, 129:257],
                                in1=rsb[:, 1:2].to_broadcast([D, D]),
                                op=mybir.AluOpType.mult,
                            )
                            nc.vector.tensor_tensor(
                                out=osb[:], in0=osb[:], in1=brep[:, 0:256],
                                op=mybir.AluOpType.add,
                            )
                            nc.scalar.dma_start(
                                out=out_d[t * 128 : (t + 1) * 128, :], in_=osb[:]
                            )

                # ---- inter-layer allgather ----
                if l == 0:
                    nc.sync.dma_start(
                        out=hT_own1[:], in_=hT_stage[:, ch0 : cfg.nloc]
                    )
                    with nc.named_scope("cc"):
                        nc.gpsimd.collective_compute(
                            "AllGather",
                            mybir.AluOpType.bypass,
                            replica_groups=[list(range(C))],
                            ins=[hT_own1[:]],
                            outs=[hT_all1[:]],
                        )
    nc.compile()
    return nc


# ----------------------------------------------------------------------------
# Entry point
# ----------------------------------------------------------------------------

def run_gat(cfg, x, Ws, als, ars, bs, src, dst, trace=False):
    geom, src_w, dst_w, dstrow = prep_edges(
        cfg, np.asarray(src), np.asarray(dst)
    )
    wc, bp = prep_weights(cfg, Ws, als, ars, bs)

    x = np.asarray(x, dtype=np.float32)
    xTb = np.ascontiguousarray(
        x.reshape(N_CORES, cfg.nloc, D).transpose(0, 2, 1)
    ).astype(np.float16)

    onehots = []
    for c in range(N_CORES):
        oh = np.zeros((D, 8), dtype=np.float32)
        oh[:, c] = 1.0
        onehots.append(oh)

    nc = build(cfg, geom)
    in_maps = []
    for c in range(N_CORES):
        in_maps.append({
            "xTb": xTb,
            "wc": wc,
            "bp": bp,
            "srcw": src_w[c],
            "dstw": dst_w[c].astype(np.float16),
            "dstrow": dstrow[c],
            "onehot": onehots[c],
        })
    res = run_bass_kernel_spmd(nc, in_maps, list(range(N_CORES)), trace=trace)
    outs = [res.results[c]["out"][: cfg.nloc] for c in range(N_CORES)]
    out = np.concatenate(outs, axis=0).reshape(cfg.n, H, D)
    return out, res


def kernel(x, Ws, als, ars, bs, src, dst):
    out, _ = run_gat(FULL, x, Ws, als, ars, bs, src, dst, trace=False)
    return out.astype(np.float32)


# revision 7
# speedup vs baseline: 1.0796x; 1.0796x over previous
"""Trainium2 Bass kernel for 2-layer GAT (nn_GAT_43765716746408).

Key design points (vs the 5.15 ms baseline):
  - No er dma_gather: per-edge er(dst) is selected on the tensor engine via a
    transposed one-hot ST[d, e] = (dst[e] == d) built from a DMA-replicated
    dst row (0-stride DRAM source), then ere = ST^T @ er2 (f16, 2 columns).
  - No 2-port DVE ops (tensor_scalar / copy / cast) concurrent with SWDGE
    descriptor generation: the DVE<->GpSimd shared SBUF port lock serialized
    the baseline edge phase. Hot-path element-wise work is tensor_tensor /
    reduce (single-port classes) or runs on the scalar engine.
  - Aggregation uses weight-scaled one-hots: Swh = S * w_h, and tbig rows
    carry a literal 1.0 column per head so the denominators fall out of the
    same matmuls. The two head chains run sequentially in one PSUM bank
    (start=True clears accumulate flags bank-wide).
  - leaky-relu on DVE as max(u, 0.2u), exp on ACT.
  - tbig is double-buffered across layers; the inter-layer AllGather is split
    in halves, and layer-1's P1 chunks for the first half are EMITTED inside
    layer-0's edge loop (engine queues are in-order FIFOs, so overlap
    requires interleaved program order).
"""

import sys

sys.path.insert(0, "/opt/trn_rl_repo")

import numpy as np

import concourse.bass as bass
import concourse.tile as tile
from concourse import bacc, mybir
from concourse.bass_utils import run_bass_kernel_spmd
from concourse.masks import make_identity

F32 = mybir.dt.float32
F16 = mybir.dt.float16
I16 = mybir.dt.int16
I32 = mybir.dt.int32

N_CORES = 8
D = 128          # model dim
H = 2            # heads
HF = 256         # H * F
ROW16 = 384      # fp16 elements per TBIG row (768B pitch)
NEG_SLOPE = 0.2

# tbig row layout (fp16 elems):
#   [0:128]   f0 (head-0 features)
#   [128]     1.0           (denominator column for head 0)
#   [129:257] f1
#   [257]     1.0           (denominator column for head 1)
#   [258:262] el0, el1 as two f32 (bitcast)
#   [262:384] pad (never read)


class Cfg:
    def __init__(self, n_nodes, n_edges, n_layers=2):
        assert n_nodes % N_CORES == 0
        self.n = n_nodes
        self.e = n_edges
        self.layers = n_layers
        self.nloc = n_nodes // N_CORES
        self.t = -(-self.nloc // 128)          # dst tiles per core
        self.nloc_pad = self.t * 128
        self.w_last = self.nloc - 128 * (self.t - 1)
        self.split = n_nodes // 2              # lo/hi table split (int16 range)
        assert self.split < 32768 and (n_nodes - self.split) < 32768


FULL = Cfg(50000, 800000)


# ----------------------------------------------------------------------------
# Host-side edge preprocessing
# ----------------------------------------------------------------------------

def prep_edges(cfg, src, dst):
    """Bucket edges per core by (dst_tile, src_half); pad to shared sizes."""
    C, T = N_CORES, cfg.t
    counts = np.zeros((C, T, 2), dtype=np.int64)
    per_core = []
    core_of = dst // cfg.nloc
    for c in range(C):
        sel = core_of == c
        es, ed = src[sel].astype(np.int64), dst[sel].astype(np.int64)
        dloc = ed - c * cfg.nloc
        t = dloc // 128
        half = (es >= cfg.split).astype(np.int64)
        order = np.lexsort((es, half, t))
        es, dloc, t, half = es[order], dloc[order], t[order], half[order]
        np.add.at(counts[c], (t, half), 1)
        per_core.append((es, dloc, t, half))

    gmax_th = counts.max(axis=0)                       # (T, 2)
    G = np.maximum(1, -(-gmax_th // 128))              # groups per (t, half)
    base = np.zeros((T, 2), dtype=np.int64)
    acc = 0
    for t in range(T):
        for h in range(2):
            base[t, h] = acc
            acc += G[t, h]
    gtot = acc
    nslot = gtot * 128

    src_idx = np.zeros((C, nslot), dtype=np.int16)
    dst_reb = np.full((C, nslot), -1.0, dtype=np.float16)
    for c in range(C):
        es, dloc, t, half = per_core[c]
        bucket_id = t * 2 + half
        n = len(es)
        starts = np.searchsorted(bucket_id, np.arange(T * 2), side="left")
        pos_in_bucket = np.arange(n) - starts[bucket_id]
        slot = base[t, half] * 128 + pos_in_bucket
        src_idx[c, slot] = (es - np.where(half == 1, cfg.split, 0)).astype(np.int16)
        dst_reb[c, slot] = (dloc - t * 128).astype(np.float32)

    def wrap16(a):
        w = a.reshape(-1, 16).T.copy()                 # (16, nslot/16)
        return np.tile(w, (8, 1))                      # (128, nslot/16)

    src_w = np.stack([wrap16(src_idx[c]) for c in range(C)])
    # dst one-hot source, partition-major per group: dst_w[c][p, g] = dst of
    # slot (p, g)
    dst_w = np.stack([dst_reb[c].reshape(-1, 128).T.copy() for c in range(C)])
    # slot-major rows per tile for the DMA-replicated ST build:
    # dstrow[c][t, g*128 + p] = dst of slot (p, g) of tile t (or -1 pad)
    gt_n = G[:, 0] + G[:, 1]
    gmax = int(gt_n.max())
    dstrow = np.full((C, T, gmax * 128), -1.0, dtype=np.float16)
    for c in range(C):
        flat = dst_reb[c]
        for t in range(T):
            g0 = int(base[t, 0])
            n_in_tile = int(gt_n[t]) * 128
            dstrow[c, t, :n_in_tile] = flat[g0 * 128 : g0 * 128 + n_in_tile]

    geom = {"G": G, "base": base, "gtot": gtot, "gmax": gmax}
    return geom, src_w, dst_w, dstrow


def prep_weights(cfg, Ws, als, ars, bs):
    """Combined matmul weights Wc = [W | wl | wr] and packed bias rows."""
    L = cfg.layers
    wc = np.zeros((L, D, HF + 4), dtype=np.float16)
    bp = np.zeros((L, 1, 384), dtype=np.float16)
    for l in range(L):
        W = np.asarray(Ws[l], dtype=np.float32)            # (D, H*F)
        Wh = W.reshape(D, H, D)
        wl = np.einsum("khf,hf->kh", Wh, np.asarray(als[l], np.float32))
        wr = np.einsum("khf,hf->kh", Wh, np.asarray(ars[l], np.float32))
        wc[l, :, :HF] = W.astype(np.float16)
        wc[l, :, HF : HF + 2] = wl.astype(np.float16)
        wc[l, :, HF + 2 : HF + 4] = wr.astype(np.float16)
        b = np.asarray(bs[l], np.float32)
        bp[l, 0, 0:128] = b[0].astype(np.float16)
        bp[l, 0, 128:256] = b[1].astype(np.float16)
        bp[l, 0, 256:384] = (0.5 * (b[0] + b[1])).astype(np.float16)
    return wc, bp


# ----------------------------------------------------------------------------
# Device kernel
# ----------------------------------------------------------------------------

def build(cfg, geom):
    C, T, L = N_CORES, cfg.t, cfg.layers
    G, base, gtot, gmax = geom["G"], geom["base"], geom["gtot"], geom["gmax"]
    nslot = gtot * 128
    NLO = cfg.split

    nc = bacc.Bacc("TRN2", target_bir_lowering=False, debug=False,
                   enable_asserts=False, num_devices=C, num_swdge_queues=2)

    # I/O
    xTb = nc.dram_tensor("xTb", [C, D, cfg.nloc], F16, kind="ExternalInput")
    wc_d = nc.dram_tensor("wc", [L, D, HF + 4], F16, kind="ExternalInput")
    bp_d = nc.dram_tensor("bp", [L, 1, 384], F16, kind="ExternalInput")
    src_d = nc.dram_tensor("srcw", [D, nslot // 16], I16, kind="ExternalInput")
    dst_d = nc.dram_tensor("dstw", [D, gtot], F16, kind="ExternalInput")
    dstrow_d = nc.dram_tensor("dstrow", [T, gmax * 128], F16, kind="ExternalInput")
    oh_d = nc.dram_tensor("onehot", [D, 8], F32, kind="ExternalInput")
    out_d = nc.dram_tensor("out", [cfg.nloc_pad, HF], F32, kind="ExternalOutput")

    # internal DRAM (tbig double-buffered across layers)
    tbig = [nc.dram_tensor(f"tbig{l}", [cfg.n, ROW16], F16) for l in range(L)]
    THALF = (T + 1) // 2
    ch0 = min(THALF * 128, cfg.nloc)
    ch1 = cfg.nloc - ch0
    hT_own0 = nc.dram_tensor("hT_own0", [D, ch0], F16)
    hT_own1 = nc.dram_tensor("hT_own1", [D, ch1], F16)
    hT_all0 = nc.dram_tensor("hT_all0", [C, D, ch0], F16)
    hT_all1 = nc.dram_tensor("hT_all1", [C, D, ch1], F16)

    def chunk_list(lo, hi):
        out = []
        t = lo
        while t < hi:
            nt = 2 if (t + 2 <= hi and t + 1 != T - 1) else 1
            out.append((t, nt))
            t += nt
        return out

    with tile.TileContext(nc) as tc:
        with (
            tc.tile_pool(name="const", bufs=1) as cpool,
            tc.tile_pool(name="work", bufs=2) as pool,
            tc.tile_pool(name="p1w", bufs=4) as p1w,
            tc.tile_pool(name="gtp", bufs=3) as gtp,
            tc.tile_pool(name="ppA", bufs=3, space="PSUM") as ppA,
            tc.tile_pool(name="ppB", bufs=1, space="PSUM") as ppB,
        ):
            # ---- constants ----
            src_sb = cpool.tile([D, nslot // 16], I16, tag="src_sb")
            nc.sync.dma_start(out=src_sb[:], in_=src_d[:])
            dst_sb = cpool.tile([D, gtot], F16, tag="dst_sb")
            nc.sync.dma_start(out=dst_sb[:], in_=dst_d[:])
            oh_sb = cpool.tile([D, 8], F32, tag="oh_sb")
            nc.sync.dma_start(out=oh_sb[:], in_=oh_d[:])
            wc_sb = cpool.tile([D, L * (HF + 4)], F16, tag="wc_sb")
            bp_sb = cpool.tile([1, L * 384], F16, tag="bp_sb")
            for l in range(L):
                nc.sync.dma_start(
                    out=wc_sb[:, l * (HF + 4) : (l + 1) * (HF + 4)], in_=wc_d[l]
                )
                nc.sync.dma_start(
                    out=bp_sb[:, l * 384 : (l + 1) * 384], in_=bp_d[l]
                )

            it16 = cpool.tile([D, D], I16, tag="it16")
            nc.gpsimd.iota(it16[:], pattern=[[1, D]], base=0, channel_multiplier=0)
            iota_f = cpool.tile([D, D], F16, tag="iota_f")
            nc.vector.tensor_copy(iota_f[:], it16[:])

            ip32 = cpool.tile([D, 1], I32, tag="ip32")
            nc.gpsimd.iota(ip32[:], pattern=[[1, 1]], base=0, channel_multiplier=1)
            iota_p16 = cpool.tile([D, 1], F16, tag="iota_p16")
            nc.vector.tensor_copy(iota_p16[:], ip32[:])

            ones_row = cpool.tile([1, D], F16, tag="ones_row")
            nc.vector.memset(ones_row[:], 1.0)
            ident = cpool.tile([D, D], F16, tag="ident")
            make_identity(nc, ident[:])

            epsc = cpool.tile([D, 1], F32, tag="epsc")
            nc.vector.memset(epsc[:], 1e-30)
            halfc = cpool.tile([D, 1], F32, tag="halfc")
            nc.vector.memset(halfc[:], 0.5)
            slopec = cpool.tile([D, 1], F32, tag="slopec")
            nc.vector.memset(slopec[:], NEG_SLOPE)
            zc32 = cpool.tile([D, 1], F32, tag="zc32")
            nc.vector.memset(zc32[:], 0.0)
            zc16 = cpool.tile([D, 1], F16, tag="zc16")
            nc.vector.memset(zc16[:], 0.0)

            # layer-indexed staging (layers overlap: tbig double-buffered)
            er_stage = cpool.tile([D, L, T, 2, 8], F32, tag="er_stage")
            hT_stage = cpool.tile([D, cfg.nloc_pad], F16, tag="hT_stage")
            brep_t = cpool.tile([D, L, 384], F32, tag="brep")

            def emit_layer_prep(l):
                brep = brep_t[:, l]
                for k in range(3):
                    bps = ppA.tile([D, HF + 4], F32, tag="p1")
                    nc.tensor.matmul(
                        bps[:, 0:128], lhsT=ones_row[:],
                        rhs=bp_sb[:, l * 384 + k * 128 : l * 384 + (k + 1) * 128],
                        start=True, stop=True,
                    )
                    nc.vector.tensor_tensor(
                        out=brep[:, k * 128 : (k + 1) * 128], in0=bps[:, 0:128],
                        in1=zc32[:, 0:1].to_broadcast([D, D]),
                        op=mybir.AluOpType.add,
                    )
                nc.vector.memset(er_stage[:, l], 0.0)

            def emit_p1_chunk(l, cb, t, nt):
                wcl = wc_sb[:, l * (HF + 4) : l * (HF + 4) + HF + 4]
                tb = tbig[l]
                ws = [cfg.w_last if t + j == T - 1 else 128 for j in range(nt)]
                w2 = sum(ws)
                xt = p1w.tile([D, 256], F16, tag="xt")
                if l == 0:
                    src_ap = xTb[cb, :, t * 128 : t * 128 + w2]
                elif t < THALF:
                    src_ap = hT_all0[cb, :, t * 128 : t * 128 + w2]
                else:
                    c0 = t * 128 - ch0
                    src_ap = hT_all1[cb, :, c0 : c0 + w2]
                nc.sync.dma_start(out=xt[:, :w2], in_=src_ap)
                stage = p1w.tile([D, 2, 384], F16, tag="stage")
                for j in range(nt):
                    wj = ws[j]
                    ps1 = ppA.tile([D, HF + 4], F32, tag="p1")
                    nc.tensor.matmul(
                        ps1[:wj, :], lhsT=xt[:, j * 128 : j * 128 + wj],
                        rhs=wcl, start=True, stop=True,
                    )
                    # f0 | f1 at cols 0:128 / 129:257 (stride 129)
                    sv = stage[:, j, 0:258].rearrange("p (h v) -> p h v", h=2)
                    nc.scalar.activation(
                        sv[:wj, :, 0:128],
                        ps1[:wj, 0:HF].rearrange("p (h v) -> p h v", h=2),
                        mybir.ActivationFunctionType.Copy,
                    )
                    nc.scalar.activation(
                        stage[:wj, j, 258:262].bitcast(F32),
                        ps1[:wj, HF : HF + 2],
                        mybir.ActivationFunctionType.Copy,
                    )
                    nc.vector.tensor_tensor(
                        out=er_stage[:wj, l, t + j, :, cb],
                        in0=ps1[:wj, HF + 2 : HF + 4],
                        in1=zc32[:wj, 0:1].to_broadcast([wj, 2]),
                        op=mybir.AluOpType.add,
                    )
                nc.vector.memset(stage[:, :, 128:129], 1.0)
                nc.vector.memset(stage[:, :, 257:258], 1.0)
                n0 = cb * cfg.nloc + t * 128
                if nt == 2:
                    tbv = tb[n0 : n0 + 256, :].rearrange("(j p) v -> p j v", p=128)
                    nc.sync.dma_start(out=tbv[:, :, 0:262], in_=stage[:, :, 0:262])
                else:
                    nc.sync.dma_start(
                        out=tb[n0 : n0 + ws[0], 0:262], in_=stage[: ws[0], 0, 0:262]
                    )

            def emit_p2_tile(l, t):
                brep = brep_t[:, l]
                tb = tbig[l]
                g_lo, g_hi = int(G[t, 0]), int(G[t, 1])
                gt_n = g_lo + g_hi
                goff = int(base[t, 0])

                gt = gtp.tile([D, gmax, ROW16], F16, tag="gt")
                nc.gpsimd.dma_gather(
                    out_ap=gt[:, 0:g_lo, :],
                    in_ap=tb[0:NLO, :],
                    idxs_ap=src_sb[:, goff * 8 : (goff + g_lo) * 8],
                    num_idxs=g_lo * 128,
                    num_idxs_reg=g_lo * 128,
                    elem_size=ROW16,
                    queue_num=0,
                    single_packet=False,
                )
                nc.gpsimd.dma_gather(
                    out_ap=gt[:, g_lo:gt_n, :],
                    in_ap=tb[NLO : cfg.n, :],
                    idxs_ap=src_sb[:, (goff + g_lo) * 8 : (goff + gt_n) * 8],
                    num_idxs=g_hi * 128,
                    num_idxs_reg=g_hi * 128,
                    elem_size=ROW16,
                    queue_num=1,
                    single_packet=False,
                )

                # er2[d, h] = own-core er of node t*128+d
                tmp8 = pool.tile([D, 2, 8], F32, tag="tmp8")
                nc.vector.tensor_tensor(
                    out=tmp8[:],
                    in0=er_stage[:, l, t],
                    in1=oh_sb[:].unsqueeze(1).to_broadcast([D, 2, 8]),
                    op=mybir.AluOpType.mult,
                )
                er2f = pool.tile([D, 2], F32, tag="er2f")
                nc.vector.reduce_sum(er2f[:], tmp8[:], axis=mybir.AxisListType.X)
                er2 = pool.tile([D, 2], F16, tag="er2")
                nc.vector.tensor_tensor(
                    out=er2[:], in0=er2f[:],
                    in1=zc32[:, 0:1].to_broadcast([D, 2]),
                    op=mybir.AluOpType.add,
                )

                # transposed one-hot from DMA-replicated dst row; ere = ST^T@er2
                drep = pool.tile([D, gmax * 128], F16, tag="drep")
                nc.sync.dma_start(
                    out=drep[:, 0 : gt_n * 128],
                    in_=dstrow_d[t : t + 1, 0 : gt_n * 128].to_broadcast(
                        [D, gt_n * 128]
                    ),
                )
                ST = pool.tile([D, gmax, D], F16, tag="ST")
                nc.vector.tensor_tensor(
                    out=ST[:, 0:gt_n],
                    in0=drep[:, 0 : gt_n * 128].rearrange("p (g e) -> p g e", e=D),
                    in1=iota_p16[:, 0:1].unsqueeze(1).to_broadcast([D, gt_n, D]),
                    op=mybir.AluOpType.is_equal,
                )
                ps_ere = ppB.tile([D, gmax, 2], F32, tag="ere")
                for g in range(gt_n):
                    nc.tensor.matmul(
                        ps_ere[:, g, :],
                        lhsT=ST[:, g, :],
                        rhs=er2[:],
                        start=True, stop=True,
                    )

                # u = el_src + er_dst ; w = exp(max(u, 0.2u))
                elv = gt[:, 0:gt_n, 258:262].bitcast(F32)
                u = pool.tile([D, gmax, 2], F32, tag="u")
                nc.vector.tensor_tensor(
                    out=u[:, 0:gt_n], in0=ps_ere[:, 0:gt_n], in1=elv,
                    op=mybir.AluOpType.add,
                )
                lr = pool.tile([D, gmax, 2], F32, tag="lr")
                nc.vector.tensor_tensor(
                    out=lr[:, 0:gt_n], in0=u[:, 0:gt_n],
                    in1=slopec[:, 0:1].unsqueeze(1).to_broadcast([D, gt_n, 2]),
                    op=mybir.AluOpType.mult,
                )
                nc.vector.tensor_tensor(
                    out=u[:, 0:gt_n], in0=u[:, 0:gt_n], in1=lr[:, 0:gt_n],
                    op=mybir.AluOpType.max,
                )
                w16 = pool.tile([D, gmax, 2], F16, tag="w16")
                nc.scalar.activation(
                    w16[:, 0:gt_n], u[:, 0:gt_n],
                    mybir.ActivationFunctionType.Exp,
                )

                # one-hot S[e, d] and per-head scaled copies
                S = pool.tile([D, gmax, D], F16, tag="S")
                nc.vector.tensor_tensor(
                    out=S[:, 0:gt_n],
                    in0=dst_sb[:, goff : goff + gt_n].unsqueeze(2).to_broadcast(
                        [D, gt_n, D]
                    ),
                    in1=iota_f[:].unsqueeze(1).to_broadcast([D, gt_n, D]),
                    op=mybir.AluOpType.is_equal,
                )
                Sw0 = pool.tile([D, gmax, D], F16, tag="Sw0")
                nc.vector.tensor_tensor(
                    out=Sw0[:, 0:gt_n],
                    in0=S[:, 0:gt_n],
                    in1=w16[:, 0:gt_n, 0:1].to_broadcast([D, gt_n, D]),
                    op=mybir.AluOpType.mult,
                )
                Sw1 = pool.tile([D, gmax, D], F16, tag="Sw1")
                nc.vector.tensor_tensor(
                    out=Sw1[:, 0:gt_n],
                    in0=S[:, 0:gt_n],
                    in1=w16[:, 0:gt_n, 1:2].to_broadcast([D, gt_n, D]),
                    op=mybir.AluOpType.mult,
                )

                # aggregation: two sequential chains sharing one bank
                # (start=True clears accumulate flags bank-wide, so the
                # chains must not interleave)
                ps_agg = ppA.tile([D, 258], F32, tag="agg")
                ps_a0 = ps_agg[:, 0:129]
                ps_a1 = ps_agg[:, 129:258]
                for g in range(gt_n):
                    nc.tensor.matmul(
                        ps_a0,
                        lhsT=Sw0[:, g, :],
                        rhs=gt[:, g, 0:129],
                        start=(g == 0),
                        stop=(g == gt_n - 1),
                    )
                for g in range(gt_n):
                    nc.tensor.matmul(
                        ps_a1,
                        lhsT=Sw1[:, g, :],
                        rhs=gt[:, g, 129:258],
                        start=(g == 0),
                        stop=(g == gt_n - 1),
                    )

                # ---- finalize ----
                rsb = pool.tile([D, 2], F32, tag="rsb")
                nc.vector.tensor_tensor(
                    out=rsb[:, 0:1], in0=ps_agg[:, 128:129], in1=epsc[:],
                    op=mybir.AluOpType.max,
                )
                nc.vector.tensor_tensor(
                    out=rsb[:, 1:2], in0=ps_agg[:, 257:258], in1=epsc[:],
                    op=mybir.AluOpType.max,
                )
                nc.vector.reciprocal(rsb[:], rsb[:])
                if l == 0:
                    rh = pool.tile([D, 2], F32, tag="rh")
                    nc.vector.tensor_tensor(
                        out=rh[:], in0=rsb[:],
                        in1=halfc[:, 0:1].to_broadcast([D, 2]),
                        op=mybir.AluOpType.mult,
                    )
                    t0 = pool.tile([D, D], F32, tag="t0")
                    nc.vector.tensor_tensor(
                        out=t0[:], in0=ps_agg[:, 0:128],
                        in1=rh[:, 0:1].to_broadcast([D, D]),
                        op=mybir.AluOpType.mult,
                    )
                    t1 = pool.tile([D, D], F32, tag="t1")
                    nc.vector.tensor_tensor(
                        out=t1[:], in0=ps_agg[:, 129:257],
                        in1=rh[:, 1:2].to_broadcast([D, D]),
                        op=mybir.AluOpType.mult,
                    )
                    nc.vector.tensor_tensor(
                        out=t0[:], in0=t0[:], in1=t1[:], op=mybir.AluOpType.add
                    )
                    ht16 = pool.tile([D, D], F16, tag="ht16")
                    nc.vector.tensor_tensor(
                        out=ht16[:], in0=t0[:], in1=brep[:, 256:384],
                        op=mybir.AluOpType.add,
                    )
                    pst = ppB.tile([D, D], F16, tag="pt")
                    nc.tensor.transpose(pst[:], ht16[:], ident[:])
                    nc.vector.tensor_tensor(
                        out=hT_stage[:, t * 128 : (t + 1) * 128],
                        in0=pst[:],
                        in1=zc16[:, 0:1].to_broadcast([D, D]),
                        op=mybir.AluOpType.add,
                    )
                    if t == THALF - 1:
                        nc.sync.dma_start(out=hT_own0[:], in_=hT_stage[:, 0:ch0])
                        with nc.named_scope("cc0"):
                            nc.gpsimd.collective_compute(
                                "AllGather",
                                mybir.AluOpType.bypass,
                                replica_groups=[list(range(C))],
                                ins=[hT_own0[:]],
                                outs=[hT_all0[:]],
                            )
                else:
                    osb = pool.tile([D, HF], F32, tag="osb")
                    nc.vector.tensor_tensor(
                        out=osb[:, 0:128], in0=ps_agg[:, 0:128],
                        in1=rsb[:, 0:1].to_broadcast([D, D]),
                        op=mybir.AluOpType.mult,
                    )
                    nc.vector.tensor_tensor(
                        out=osb[:, 128:256], in0=ps_agg[:, 129:257],
                        in1=rsb[:, 1:2].to_broadcast([D, D]),
                        op=mybir.AluOpType.mult,
                    )
                    nc.vector.tensor_tensor(
                        out=osb[:], in0=osb[:], in1=brep[:, 0:256],
                        op=mybir.AluOpType.add,
                    )
                    nc.scalar.dma_start(
                        out=out_d[t * 128 : (t + 1) * 128, :], in_=osb[:]
                    )

            # ================= emission schedule =================
            emit_layer_prep(0)
            with nc.named_scope("p1_l0"):
                for cb in range(C):
                    for (t, nt) in chunk_list(0, T):
                        emit_p1_chunk(0, cb, t, nt)
            emit_layer_prep(1)

            # layer-1 P1 chunks for the first hT half, interleaved into the
            # second half of layer-0's edge loop (after cc0 fires)
            l1h0 = [(cb, t, nt) for cb in range(C) for (t, nt) in chunk_list(0, THALF)]
            idx = 0
            with nc.named_scope("p2_l0"):
                for t in range(T):
                    emit_p2_tile(0, t)
                    if t >= THALF:
                        quota = -(-(len(l1h0) - idx) // (T - t))
                        for _ in range(quota):
                            if idx < len(l1h0):
                                cb_, t_, nt_ = l1h0[idx]
                                with nc.named_scope("p1_l1"):
                                    emit_p1_chunk(1, cb_, t_, nt_)
                                idx += 1
            while idx < len(l1h0):
                cb_, t_, nt_ = l1h0[idx]
                with nc.named_scope("p1_l1"):
                    emit_p1_chunk(1, cb_, t_, nt_)
                idx += 1

            nc.sync.dma_start(out=hT_own1[:], in_=hT_stage[:, ch0 : cfg.nloc])
            with nc.named_scope("cc"):
                nc.gpsimd.collective_compute(
                    "AllGather",
                    mybir.AluOpType.bypass,
                    replica_groups=[list(range(C))],
                    ins=[hT_own1[:]],
                    outs=[hT_all1[:]],
                )
            with nc.named_scope("p1_l1"):
                for cb in range(C):
                    for (t, nt) in chunk_list(THALF, T):
                        emit_p1_chunk(1, cb, t, nt)
            with nc.named_scope("p2_l1"):
                for t in range(T):
                    emit_p2_tile(1, t)
    nc.compile()
    return nc


# ----------------------------------------------------------------------------
# Entry point
# ----------------------------------------------------------------------------

def run_gat(cfg, x, Ws, als, ars, bs, src, dst, trace=False):
    geom, src_w, dst_w, dstrow = prep_edges(cfg, np.asarray(src), np.asarray(dst))
    wc, bp = prep_weights(cfg, Ws, als, ars, bs)

    x = np.asarray(x, dtype=np.float32)
    xTb = np.ascontiguousarray(
        x.reshape(N_CORES, cfg.nloc, D).transpose(0, 2, 1)
    ).astype(np.float16)

    onehots = []
    for c in range(N_CORES):
        oh = np.zeros((D, 8), dtype=np.float32)
        oh[:, c] = 1.0
        onehots.append(oh)

    nc = build(cfg, geom)
    in_maps = []
    for c in range(N_CORES):
        in_maps.append({
            "xTb": xTb,
            "wc": wc,
            "bp": bp,
            "srcw": src_w[c],
            "dstw": dst_w[c].astype(np.float16),
            "dstrow": dstrow[c],
            "onehot": onehots[c],
        })
    res = run_bass_kernel_spmd(nc, in_maps, list(range(N_CORES)), trace=trace)
    outs = [res.results[c]["out"][: cfg.nloc] for c in range(N_CORES)]
    out = np.concatenate(outs, axis=0).reshape(cfg.n, H, D)
    return out, res


def kernel(x, Ws, als, ars, bs, src, dst):
    out, _ = run_gat(FULL, x, Ws, als, ars, bs, src, dst, trace=False)
    return out.astype(np.float32)


# revision 8
# speedup vs baseline: 1.0803x; 1.0006x over previous
"""Trainium2 Bass kernel for 2-layer GAT (nn_GAT_43765716746408).

Key design points (vs the 5.15 ms baseline):
  - No er dma_gather: per-edge er(dst) is selected on the tensor engine via a
    transposed one-hot ST[d, e] = (dst[e] == d) built from a DMA-replicated
    dst row (0-stride DRAM source), then ere = ST^T @ er2 (f16, 2 columns).
  - No 2-port DVE ops (tensor_scalar / copy / cast) concurrent with SWDGE
    descriptor generation: the DVE<->GpSimd shared SBUF port lock serialized
    the baseline edge phase. Hot-path element-wise work is tensor_tensor /
    reduce (single-port classes) or runs on the scalar engine.
  - Aggregation uses weight-scaled one-hots: Swh = S * w_h, and tbig rows
    carry a literal 1.0 column per head so the denominators fall out of the
    same matmuls. The two head chains run sequentially in one PSUM bank
    (start=True clears accumulate flags bank-wide).
  - leaky-relu on DVE as max(u, 0.2u), exp on ACT.
  - tbig is double-buffered across layers; the inter-layer AllGather is split
    in halves, and layer-1's P1 chunks for the first half are EMITTED inside
    layer-0's edge loop (engine queues are in-order FIFOs, so overlap
    requires interleaved program order).
"""

import sys

sys.path.insert(0, "/opt/trn_rl_repo")

import numpy as np

import concourse.bass as bass
import concourse.tile as tile
from concourse import bacc, mybir
from concourse.bass_utils import run_bass_kernel_spmd
from concourse.masks import make_identity

F32 = mybir.dt.float32
F16 = mybir.dt.float16
I16 = mybir.dt.int16
I32 = mybir.dt.int32

N_CORES = 8
D = 128          # model dim
H = 2            # heads
HF = 256         # H * F
ROW16 = 384      # fp16 elements per TBIG row (768B pitch)
NEG_SLOPE = 0.2

# tbig row layout (fp16 elems):
#   [0:128]   f0 (head-0 features)
#   [128]     1.0           (denominator column for head 0)
#   [129:257] f1
#   [257]     1.0           (denominator column for head 1)
#   [258:262] el0, el1 as two f32 (bitcast)
#   [262:384] pad (never read)


class Cfg:
    def __init__(self, n_nodes, n_edges, n_layers=2):
        assert n_nodes % N_CORES == 0
        self.n = n_nodes
        self.e = n_edges
        self.layers = n_layers
        self.nloc = n_nodes // N_CORES
        self.t = -(-self.nloc // 128)          # dst tiles per core
        self.nloc_pad = self.t * 128
        self.w_last = self.nloc - 128 * (self.t - 1)
        self.split = n_nodes // 2              # lo/hi table split (int16 range)
        assert self.split < 32768 and (n_nodes - self.split) < 32768


FULL = Cfg(50000, 800000)


# ----------------------------------------------------------------------------
# Host-side edge preprocessing
# ----------------------------------------------------------------------------

def prep_edges(cfg, src, dst):
    """Bucket edges per core by (dst_tile, src_half); pad to shared sizes."""
    C, T = N_CORES, cfg.t
    counts = np.zeros((C, T, 2), dtype=np.int64)
    per_core = []
    core_of = dst // cfg.nloc
    for c in range(C):
        sel = core_of == c
        es, ed = src[sel].astype(np.int64), dst[sel].astype(np.int64)
        dloc = ed - c * cfg.nloc
        t = dloc // 128
        half = (es >= cfg.split).astype(np.int64)
        order = np.lexsort((es, half, t))
        es, dloc, t, half = es[order], dloc[order], t[order], half[order]
        np.add.at(counts[c], (t, half), 1)
        per_core.append((es, dloc, t, half))

    gmax_th = counts.max(axis=0)                       # (T, 2)
    G = np.maximum(1, -(-gmax_th // 128))              # groups per (t, half)
    base = np.zeros((T, 2), dtype=np.int64)
    acc = 0
    for t in range(T):
        for h in range(2):
            base[t, h] = acc
            acc += G[t, h]
    gtot = acc
    nslot = gtot * 128

    src_idx = np.zeros((C, nslot), dtype=np.int16)
    dst_reb = np.full((C, nslot), -1.0, dtype=np.float16)
    for c in range(C):
        es, dloc, t, half = per_core[c]
        bucket_id = t * 2 + half
        n = len(es)
        starts = np.searchsorted(bucket_id, np.arange(T * 2), side="left")
        pos_in_bucket = np.arange(n) - starts[bucket_id]
        slot = base[t, half] * 128 + pos_in_bucket
        src_idx[c, slot] = (es - np.where(half == 1, cfg.split, 0)).astype(np.int16)
        dst_reb[c, slot] = (dloc - t * 128).astype(np.float32)

    def wrap16(a):
        w = a.reshape(-1, 16).T.copy()                 # (16, nslot/16)
        return np.tile(w, (8, 1))                      # (128, nslot/16)

    src_w = np.stack([wrap16(src_idx[c]) for c in range(C)])
    # dst one-hot source, partition-major per group: dst_w[c][p, g] = dst of
    # slot (p, g)
    dst_w = np.stack([dst_reb[c].reshape(-1, 128).T.copy() for c in range(C)])
    # slot-major rows per tile for the DMA-replicated ST build:
    # dstrow[c][t, g*128 + p] = dst of slot (p, g) of tile t (or -1 pad)
    gt_n = G[:, 0] + G[:, 1]
    gmax = int(gt_n.max())
    dstrow = np.full((C, T, gmax * 128), -1.0, dtype=np.float16)
    for c in range(C):
        flat = dst_reb[c]
        for t in range(T):
            g0 = int(base[t, 0])
            n_in_tile = int(gt_n[t]) * 128
            dstrow[c, t, :n_in_tile] = flat[g0 * 128 : g0 * 128 + n_in_tile]

    geom = {"G": G, "base": base, "gtot": gtot, "gmax": gmax}
    return geom, src_w, dst_w, dstrow


def prep_weights(cfg, Ws, als, ars, bs):
    """Combined matmul weights Wc = [W | wl | wr] and packed bias rows."""
    L = cfg.layers
    wc = np.zeros((L, D, HF + 4), dtype=np.float16)
    bp = np.zeros((L, 1, 384), dtype=np.float16)
    for l in range(L):
        W = np.asarray(Ws[l], dtype=np.float32)            # (D, H*F)
        Wh = W.reshape(D, H, D)
        wl = np.einsum("khf,hf->kh", Wh, np.asarray(als[l], np.float32))
        wr = np.einsum("khf,hf->kh", Wh, np.asarray(ars[l], np.float32))
        wc[l, :, :HF] = W.astype(np.float16)
        wc[l, :, HF : HF + 2] = wl.astype(np.float16)
        wc[l, :, HF + 2 : HF + 4] = wr.astype(np.float16)
        b = np.asarray(bs[l], np.float32)
        bp[l, 0, 0:128] = b[0].astype(np.float16)
        bp[l, 0, 128:256] = b[1].astype(np.float16)
        bp[l, 0, 256:384] = (0.5 * (b[0] + b[1])).astype(np.float16)
    return wc, bp


# ----------------------------------------------------------------------------
# Device kernel
# ----------------------------------------------------------------------------

def build(cfg, geom):
    C, T, L = N_CORES, cfg.t, cfg.layers
    G, base, gtot, gmax = geom["G"], geom["base"], geom["gtot"], geom["gmax"]
    nslot = gtot * 128
    NLO = cfg.split

    nc = bacc.Bacc("TRN2", target_bir_lowering=False, debug=False,
                   enable_asserts=False, num_devices=C, num_swdge_queues=2)

    # I/O
    xTb = nc.dram_tensor("xTb", [C, D, cfg.nloc], F16, kind="ExternalInput")
    wc_d = nc.dram_tensor("wc", [L, D, HF + 4], F16, kind="ExternalInput")
    bp_d = nc.dram_tensor("bp", [L, 1, 384], F16, kind="ExternalInput")
    src_d = nc.dram_tensor("srcw", [D, nslot // 16], I16, kind="ExternalInput")
    dst_d = nc.dram_tensor("dstw", [D, gtot], F16, kind="ExternalInput")
    dstrow_d = nc.dram_tensor("dstrow", [T, gmax * 128], F16, kind="ExternalInput")
    oh_d = nc.dram_tensor("onehot", [D, 8], F32, kind="ExternalInput")
    out_d = nc.dram_tensor("out", [cfg.nloc_pad, HF], F32, kind="ExternalOutput")

    # internal DRAM (tbig double-buffered across layers)
    tbig = [nc.dram_tensor(f"tbig{l}", [cfg.n, ROW16], F16) for l in range(L)]
    THALF = (T + 1) // 2
    ch0 = min(THALF * 128, cfg.nloc)
    ch1 = cfg.nloc - ch0
    hT_own0 = nc.dram_tensor("hT_own0", [D, ch0], F16)
    hT_own1 = nc.dram_tensor("hT_own1", [D, ch1], F16)
    hT_all0 = nc.dram_tensor("hT_all0", [C, D, ch0], F16)
    hT_all1 = nc.dram_tensor("hT_all1", [C, D, ch1], F16)

    def chunk_list(lo, hi):
        out = []
        t = lo
        while t < hi:
            nt = 2 if (t + 2 <= hi and t + 1 != T - 1) else 1
            out.append((t, nt))
            t += nt
        return out

    with tile.TileContext(nc) as tc:
        with (
            tc.tile_pool(name="const", bufs=1) as cpool,
            tc.tile_pool(name="work", bufs=2) as pool,
            tc.tile_pool(name="p1w", bufs=4) as p1w,
            tc.tile_pool(name="gtp", bufs=3) as gtp,
            tc.tile_pool(name="ppA", bufs=3, space="PSUM") as ppA,
            tc.tile_pool(name="ppB", bufs=1, space="PSUM") as ppB,
        ):
            # ---- constants ----
            src_sb = cpool.tile([D, nslot // 16], I16, tag="src_sb")
            nc.sync.dma_start(out=src_sb[:], in_=src_d[:])
            dst_sb = cpool.tile([D, gtot], F16, tag="dst_sb")
            nc.sync.dma_start(out=dst_sb[:], in_=dst_d[:])
            oh_sb = cpool.tile([D, 8], F32, tag="oh_sb")
            nc.sync.dma_start(out=oh_sb[:], in_=oh_d[:])
            wc_sb = cpool.tile([D, L * (HF + 4)], F16, tag="wc_sb")
            bp_sb = cpool.tile([1, L * 384], F16, tag="bp_sb")
            for l in range(L):
                nc.sync.dma_start(
                    out=wc_sb[:, l * (HF + 4) : (l + 1) * (HF + 4)], in_=wc_d[l]
                )
                nc.sync.dma_start(
                    out=bp_sb[:, l * 384 : (l + 1) * 384], in_=bp_d[l]
                )

            it16 = cpool.tile([D, D], I16, tag="it16")
            nc.gpsimd.iota(it16[:], pattern=[[1, D]], base=0, channel_multiplier=0)
            iota_f = cpool.tile([D, D], F16, tag="iota_f")
            nc.vector.tensor_copy(iota_f[:], it16[:])

            ip32 = cpool.tile([D, 1], I32, tag="ip32")
            nc.gpsimd.iota(ip32[:], pattern=[[1, 1]], base=0, channel_multiplier=1)
            iota_p16 = cpool.tile([D, 1], F16, tag="iota_p16")
            nc.vector.tensor_copy(iota_p16[:], ip32[:])

            ones_row = cpool.tile([1, D], F16, tag="ones_row")
            nc.vector.memset(ones_row[:], 1.0)
            ident = cpool.tile([D, D], F16, tag="ident")
            make_identity(nc, ident[:])

            epsc = cpool.tile([D, 1], F32, tag="epsc")
            nc.vector.memset(epsc[:], 1e-30)
            halfc = cpool.tile([D, 1], F32, tag="halfc")
            nc.vector.memset(halfc[:], 0.5)
            slopec = cpool.tile([D, 1], F32, tag="slopec")
            nc.vector.memset(slopec[:], NEG_SLOPE)
            zc32 = cpool.tile([D, 1], F32, tag="zc32")
            nc.vector.memset(zc32[:], 0.0)
            zc16 = cpool.tile([D, 1], F16, tag="zc16")
            nc.vector.memset(zc16[:], 0.0)
            onesc = cpool.tile([D, 1], F16, tag="onesc")
            nc.vector.memset(onesc[:], 1.0)

            # layer-indexed staging (layers overlap: tbig double-buffered)
            er_stage = cpool.tile([D, L, T, 2, 8], F32, tag="er_stage")
            hT_stage = cpool.tile([D, cfg.nloc_pad], F16, tag="hT_stage")
            brep_t = cpool.tile([D, L, 384], F32, tag="brep")

            def emit_layer_prep(l):
                brep = brep_t[:, l]
                for k in range(3):
                    bps = ppA.tile([D, HF + 4], F32, tag="p1")
                    nc.tensor.matmul(
                        bps[:, 0:128], lhsT=ones_row[:],
                        rhs=bp_sb[:, l * 384 + k * 128 : l * 384 + (k + 1) * 128],
                        start=True, stop=True,
                    )
                    nc.vector.tensor_tensor(
                        out=brep[:, k * 128 : (k + 1) * 128], in0=bps[:, 0:128],
                        in1=zc32[:, 0:1].to_broadcast([D, D]),
                        op=mybir.AluOpType.add,
                    )
                nc.vector.memset(er_stage[:, l], 0.0)

            def emit_p1_chunk(l, cb, t, nt):
                wcl = wc_sb[:, l * (HF + 4) : l * (HF + 4) + HF + 4]
                tb = tbig[l]
                ws = [cfg.w_last if t + j == T - 1 else 128 for j in range(nt)]
                w2 = sum(ws)
                xt = p1w.tile([D, 256], F16, tag="xt")
                if l == 0:
                    src_ap = xTb[cb, :, t * 128 : t * 128 + w2]
                elif t < THALF:
                    src_ap = hT_all0[cb, :, t * 128 : t * 128 + w2]
                else:
                    c0 = t * 128 - ch0
                    src_ap = hT_all1[cb, :, c0 : c0 + w2]
                nc.sync.dma_start(out=xt[:, :w2], in_=src_ap)
                stage = p1w.tile([D, 2, 384], F16, tag="stage")
                for j in range(nt):
                    wj = ws[j]
                    ps1 = ppA.tile([D, HF + 4], F32, tag="p1")
                    nc.tensor.matmul(
                        ps1[:wj, :], lhsT=xt[:, j * 128 : j * 128 + wj],
                        rhs=wcl, start=True, stop=True,
                    )
                    # f0 | f1 at cols 0:128 / 129:257 (stride 129)
                    sv = stage[:, j, 0:258].rearrange("p (h v) -> p h v", h=2)
                    nc.scalar.activation(
                        sv[:wj, :, 0:128],
                        ps1[:wj, 0:HF].rearrange("p (h v) -> p h v", h=2),
                        mybir.ActivationFunctionType.Copy,
                    )
                    nc.scalar.activation(
                        stage[:wj, j, 258:262].bitcast(F32),
                        ps1[:wj, HF : HF + 2],
                        mybir.ActivationFunctionType.Copy,
                    )
                    nc.vector.tensor_tensor(
                        out=er_stage[:wj, l, t + j, :, cb],
                        in0=ps1[:wj, HF + 2 : HF + 4],
                        in1=zc32[:wj, 0:1].to_broadcast([wj, 2]),
                        op=mybir.AluOpType.add,
                    )
                for col in (128, 257):
                    nc.vector.tensor_tensor(
                        out=stage[:, :, col : col + 1],
                        in0=onesc[:, 0:1].unsqueeze(1).to_broadcast([D, 2, 1]),
                        in1=zc16[:, 0:1].unsqueeze(1).to_broadcast([D, 2, 1]),
                        op=mybir.AluOpType.add,
                    )
                n0 = cb * cfg.nloc + t * 128
                if nt == 2:
                    tbv = tb[n0 : n0 + 256, :].rearrange("(j p) v -> p j v", p=128)
                    nc.sync.dma_start(out=tbv[:, :, 0:262], in_=stage[:, :, 0:262])
                else:
                    nc.sync.dma_start(
                        out=tb[n0 : n0 + ws[0], 0:262], in_=stage[: ws[0], 0, 0:262]
                    )

            def emit_p2_tile(l, t):
                brep = brep_t[:, l]
                tb = tbig[l]
                g_lo, g_hi = int(G[t, 0]), int(G[t, 1])
                gt_n = g_lo + g_hi
                goff = int(base[t, 0])

                gt = gtp.tile([D, gmax, ROW16], F16, tag="gt")
                nc.gpsimd.dma_gather(
                    out_ap=gt[:, 0:g_lo, :],
                    in_ap=tb[0:NLO, :],
                    idxs_ap=src_sb[:, goff * 8 : (goff + g_lo) * 8],
                    num_idxs=g_lo * 128,
                    num_idxs_reg=g_lo * 128,
                    elem_size=ROW16,
                    queue_num=0,
                    single_packet=False,
                )
                nc.gpsimd.dma_gather(
                    out_ap=gt[:, g_lo:gt_n, :],
                    in_ap=tb[NLO : cfg.n, :],
                    idxs_ap=src_sb[:, (goff + g_lo) * 8 : (goff + gt_n) * 8],
                    num_idxs=g_hi * 128,
                    num_idxs_reg=g_hi * 128,
                    elem_size=ROW16,
                    queue_num=1,
                    single_packet=False,
                )

                # er2[d, h] = own-core er of node t*128+d
                tmp8 = pool.tile([D, 2, 8], F32, tag="tmp8")
                nc.vector.tensor_tensor(
                    out=tmp8[:],
                    in0=er_stage[:, l, t],
                    in1=oh_sb[:].unsqueeze(1).to_broadcast([D, 2, 8]),
                    op=mybir.AluOpType.mult,
                )
                er2f = pool.tile([D, 2], F32, tag="er2f")
                nc.vector.reduce_sum(er2f[:], tmp8[:], axis=mybir.AxisListType.X)
                er2 = pool.tile([D, 2], F16, tag="er2")
                nc.vector.tensor_tensor(
                    out=er2[:], in0=er2f[:],
                    in1=zc32[:, 0:1].to_broadcast([D, 2]),
                    op=mybir.AluOpType.add,
                )

                # transposed one-hot from DMA-replicated dst row; ere = ST^T@er2
                drep = pool.tile([D, gmax * 128], F16, tag="drep")
                nc.sync.dma_start(
                    out=drep[:, 0 : gt_n * 128],
                    in_=dstrow_d[t : t + 1, 0 : gt_n * 128].to_broadcast(
                        [D, gt_n * 128]
                    ),
                )
                ST = pool.tile([D, gmax, D], F16, tag="ST")
                nc.vector.tensor_tensor(
                    out=ST[:, 0:gt_n],
                    in0=drep[:, 0 : gt_n * 128].rearrange("p (g e) -> p g e", e=D),
                    in1=iota_p16[:, 0:1].unsqueeze(1).to_broadcast([D, gt_n, D]),
                    op=mybir.AluOpType.is_equal,
                )
                ps_ere = ppB.tile([D, gmax, 2], F32, tag="ere")
                for g in range(gt_n):
                    nc.tensor.matmul(
                        ps_ere[:, g, :],
                        lhsT=ST[:, g, :],
                        rhs=er2[:],
                        start=True, stop=True,
                    )

                # u = el_src + er_dst ; w = exp(max(u, 0.2u))
                elv = gt[:, 0:gt_n, 258:262].bitcast(F32)
                u = pool.tile([D, gmax, 2], F32, tag="u")
                nc.vector.tensor_tensor(
                    out=u[:, 0:gt_n], in0=ps_ere[:, 0:gt_n], in1=elv,
                    op=mybir.AluOpType.add,
                )
                lr = pool.tile([D, gmax, 2], F32, tag="lr")
                nc.vector.tensor_tensor(
                    out=lr[:, 0:gt_n], in0=u[:, 0:gt_n],
                    in1=slopec[:, 0:1].unsqueeze(1).to_broadcast([D, gt_n, 2]),
                    op=mybir.AluOpType.mult,
                )
                nc.vector.tensor_tensor(
                    out=u[:, 0:gt_n], in0=u[:, 0:gt_n], in1=lr[:, 0:gt_n],
                    op=mybir.AluOpType.max,
                )
                w16 = pool.tile([D, gmax, 2], F16, tag="w16")
                nc.scalar.activation(
                    w16[:, 0:gt_n], u[:, 0:gt_n],
                    mybir.ActivationFunctionType.Exp,
                )

                # one-hot S[e, d] and per-head scaled copies
                S = pool.tile([D, gmax, D], F16, tag="S")
                nc.vector.tensor_tensor(
                    out=S[:, 0:gt_n],
                    in0=dst_sb[:, goff : goff + gt_n].unsqueeze(2).to_broadcast(
                        [D, gt_n, D]
                    ),
                    in1=iota_f[:].unsqueeze(1).to_broadcast([D, gt_n, D]),
                    op=mybir.AluOpType.is_equal,
                )
                Sw0 = pool.tile([D, gmax, D], F16, tag="Sw0")
                nc.vector.tensor_tensor(
                    out=Sw0[:, 0:gt_n],
                    in0=S[:, 0:gt_n],
                    in1=w16[:, 0:gt_n, 0:1].to_broadcast([D, gt_n, D]),
                    op=mybir.AluOpType.mult,
                )
                Sw1 = pool.tile([D, gmax, D], F16, tag="Sw1")
                nc.vector.tensor_tensor(
                    out=Sw1[:, 0:gt_n],
                    in0=S[:, 0:gt_n],
                    in1=w16[:, 0:gt_n, 1:2].to_broadcast([D, gt_n, D]),
                    op=mybir.AluOpType.mult,
                )

                # aggregation: two sequential chains sharing one bank
                # (start=True clears accumulate flags bank-wide, so the
                # chains must not interleave)
                ps_agg = ppA.tile([D, 258], F32, tag="agg")
                ps_a0 = ps_agg[:, 0:129]
                ps_a1 = ps_agg[:, 129:258]
                for g in range(gt_n):
                    nc.tensor.matmul(
                        ps_a0,
                        lhsT=Sw0[:, g, :],
                        rhs=gt[:, g, 0:129],
                        start=(g == 0),
                        stop=(g == gt_n - 1),
                    )
                for g in range(gt_n):
                    nc.tensor.matmul(
                        ps_a1,
                        lhsT=Sw1[:, g, :],
                        rhs=gt[:, g, 129:258],
                        start=(g == 0),
                        stop=(g == gt_n - 1),
                    )

                # ---- finalize ----
                rsb = pool.tile([D, 2], F32, tag="rsb")
                nc.vector.tensor_tensor(
                    out=rsb[:, 0:1], in0=ps_agg[:, 128:129], in1=epsc[:],
                    op=mybir.AluOpType.max,
                )
                nc.vector.tensor_tensor(
                    out=rsb[:, 1:2], in0=ps_agg[:, 257:258], in1=epsc[:],
                    op=mybir.AluOpType.max,
                )
                nc.vector.reciprocal(rsb[:], rsb[:])
                if l == 0:
                    rh = pool.tile([D, 2], F32, tag="rh")
                    nc.vector.tensor_tensor(
                        out=rh[:], in0=rsb[:],
                        in1=halfc[:, 0:1].to_broadcast([D, 2]),
                        op=mybir.AluOpType.mult,
                    )
                    t0 = pool.tile([D, D], F32, tag="t0")
                    nc.vector.tensor_tensor(
                        out=t0[:], in0=ps_agg[:, 0:128],
                        in1=rh[:, 0:1].to_broadcast([D, D]),
                        op=mybir.AluOpType.mult,
                    )
                    t1 = pool.tile([D, D], F32, tag="t1")
                    nc.vector.tensor_tensor(
                        out=t1[:], in0=ps_agg[:, 129:257],
                        in1=rh[:, 1:2].to_broadcast([D, D]),
                        op=mybir.AluOpType.mult,
                    )
                    nc.vector.tensor_tensor(
                        out=t0[:], in0=t0[:], in1=t1[:], op=mybir.AluOpType.add
                    )
                    ht16 = pool.tile([D, D], F16, tag="ht16")
                    nc.vector.tensor_tensor(
                        out=ht16[:], in0=t0[:], in1=brep[:, 256:384],
                        op=mybir.AluOpType.add,
                    )
                    pst = ppB.tile([D, D], F16, tag="pt")
                    nc.tensor.transpose(pst[:], ht16[:], ident[:])
                    nc.vector.tensor_tensor(
                        out=hT_stage[:, t * 128 : (t + 1) * 128],
                        in0=pst[:],
                        in1=zc16[:, 0:1].to_broadcast([D, D]),
                        op=mybir.AluOpType.add,
                    )
                    if t == THALF - 1:
                        nc.sync.dma_start(out=hT_own0[:], in_=hT_stage[:, 0:ch0])
                        with nc.named_scope("cc0"):
                            nc.gpsimd.collective_compute(
                                "AllGather",
                                mybir.AluOpType.bypass,
                                replica_groups=[list(range(C))],
                                ins=[hT_own0[:]],
                                outs=[hT_all0[:]],
                            )
                else:
                    osb = pool.tile([D, HF], F32, tag="osb")
                    nc.vector.tensor_tensor(
                        out=osb[:, 0:128], in0=ps_agg[:, 0:128],
                        in1=rsb[:, 0:1].to_broadcast([D, D]),
                        op=mybir.AluOpType.mult,
                    )
                    nc.vector.tensor_tensor(
                        out=osb[:, 128:256], in0=ps_agg[:, 129:257],
                        in1=rsb[:, 1:2].to_broadcast([D, D]),
                        op=mybir.AluOpType.mult,
                    )
                    nc.vector.tensor_tensor(
                        out=osb[:], in0=osb[:], in1=brep[:, 0:256],
                        op=mybir.AluOpType.add,
                    )
                    nc.scalar.dma_start(
                        out=out_d[t * 128 : (t + 1) * 128, :], in_=osb[:]
                    )

            # ================= emission schedule =================
            emit_layer_prep(0)
            with nc.named_scope("p1_l0"):
                for cb in range(C):
                    for (t, nt) in chunk_list(0, T):
                        emit_p1_chunk(0, cb, t, nt)
            emit_layer_prep(1)

            # layer-1 P1 chunks for the first hT half, interleaved into the
            # second half of layer-0's edge loop (after cc0 fires)
            l1h0 = [(cb, t, nt) for cb in range(C) for (t, nt) in chunk_list(0, THALF)]
            idx = 0
            with nc.named_scope("p2_l0"):
                for t in range(T):
                    emit_p2_tile(0, t)
                    if t >= THALF:
                        quota = -(-(len(l1h0) - idx) // (T - t))
                        for _ in range(quota):
                            if idx < len(l1h0):
                                cb_, t_, nt_ = l1h0[idx]
                                with nc.named_scope("p1_l1"):
                                    emit_p1_chunk(1, cb_, t_, nt_)
                                idx += 1
            while idx < len(l1h0):
                cb_, t_, nt_ = l1h0[idx]
                with nc.named_scope("p1_l1"):
                    emit_p1_chunk(1, cb_, t_, nt_)
                idx += 1

            nc.sync.dma_start(out=hT_own1[:], in_=hT_stage[:, ch0 : cfg.nloc])
            with nc.named_scope("cc"):
                nc.gpsimd.collective_compute(
                    "AllGather",
                    mybir.AluOpType.bypass,
                    replica_groups=[list(range(C))],
                    ins=[hT_own1[:]],
                    outs=[hT_all1[:]],
                )
            with nc.named_scope("p1_l1"):
                for cb in range(C):
                    for (t, nt) in chunk_list(THALF, T):
                        emit_p1_chunk(1, cb, t, nt)
            with nc.named_scope("p2_l1"):
                for t in range(T):
                    emit_p2_tile(1, t)
    nc.compile()
    return nc


# ----------------------------------------------------------------------------
# Entry point
# ----------------------------------------------------------------------------

def run_gat(cfg, x, Ws, als, ars, bs, src, dst, trace=False):
    geom, src_w, dst_w, dstrow = prep_edges(cfg, np.asarray(src), np.asarray(dst))
    wc, bp = prep_weights(cfg, Ws, als, ars, bs)

    x = np.asarray(x, dtype=np.float32)
    xTb = np.ascontiguousarray(
        x.reshape(N_CORES, cfg.nloc, D).transpose(0, 2, 1)
    ).astype(np.float16)

    onehots = []
    for c in range(N_CORES):
        oh = np.zeros((D, 8), dtype=np.float32)
        oh[:, c] = 1.0
        onehots.append(oh)

    nc = build(cfg, geom)
    in_maps = []
    for c in range(N_CORES):
        in_maps.append({
            "xTb": xTb,
            "wc": wc,
            "bp": bp,
            "srcw": src_w[c],
            "dstw": dst_w[c].astype(np.float16),
            "dstrow": dstrow[c],
            "onehot": onehots[c],
        })
    res = run_bass_kernel_spmd(nc, in_maps, list(range(N_CORES)), trace=trace)
    outs = [res.results[c]["out"][: cfg.nloc] for c in range(N_CORES)]
    out = np.concatenate(outs, axis=0).reshape(cfg.n, H, D)
    return out, res


def kernel(x, Ws, als, ars, bs, src, dst):
    out, _ = run_gat(FULL, x, Ws, als, ars, bs, src, dst, trace=False)
    return out.astype(np.float32)


# revision 9
# speedup vs baseline: 1.1190x; 1.0358x over previous
"""Trainium2 Bass kernel for 2-layer GAT (nn_GAT_43765716746408).

Key design points (vs the 5.15 ms baseline):
  - No er dma_gather: per-edge er(dst) is selected on the tensor engine via a
    transposed one-hot ST[d, e] = (dst[e] == d) built from a DMA-replicated
    dst row (0-stride DRAM source), then ere = ST^T @ er2 (f16, 2 columns).
  - No 2-port DVE ops (tensor_scalar / copy / cast) concurrent with SWDGE
    descriptor generation: the DVE<->GpSimd shared SBUF port lock serialized
    the baseline edge phase. Hot-path element-wise work is tensor_tensor /
    reduce (single-port classes) or runs on the scalar engine.
  - Aggregation uses weight-scaled one-hots: Swh = S * w_h, and tbig rows
    carry a literal 1.0 column per head so the denominators fall out of the
    same matmuls. The two head chains run sequentially in one PSUM bank
    (start=True clears accumulate flags bank-wide).
  - leaky-relu on DVE as max(u, 0.2u), exp on ACT.
  - tbig is double-buffered across layers; the inter-layer AllGather is split
    in halves, and layer-1's P1 chunks for the first half are EMITTED inside
    layer-0's edge loop (engine queues are in-order FIFOs, so overlap
    requires interleaved program order).
"""

import sys

sys.path.insert(0, "/opt/trn_rl_repo")

import numpy as np

import concourse.bass as bass
import concourse.tile as tile
from concourse import bacc, mybir
from concourse.bass_utils import run_bass_kernel_spmd
from concourse.masks import make_identity

F32 = mybir.dt.float32
F16 = mybir.dt.float16
I8 = mybir.dt.int8
I16 = mybir.dt.int16
I32 = mybir.dt.int32

N_CORES = 8
D = 128          # model dim
H = 2            # heads
HF = 256         # H * F
ROW16 = 384      # fp16 elements per TBIG row (768B pitch)
NEG_SLOPE = 0.2

# tbig row layout (fp16 elems):
#   [0:128]   f0 (head-0 features)
#   [128]     1.0           (denominator column for head 0)
#   [129:257] f1
#   [257]     1.0           (denominator column for head 1)
#   [258:262] el0, el1 as two f32 (bitcast)
#   [262:384] pad (never read)


class Cfg:
    def __init__(self, n_nodes, n_edges, n_layers=2):
        assert n_nodes % N_CORES == 0
        self.n = n_nodes
        self.e = n_edges
        self.layers = n_layers
        self.nloc = n_nodes // N_CORES
        self.t = -(-self.nloc // 128)          # dst tiles per core
        self.nloc_pad = self.t * 128
        self.w_last = self.nloc - 128 * (self.t - 1)
        self.split = n_nodes // 2              # lo/hi table split (int16 range)
        assert self.split < 32768 and (n_nodes - self.split) < 32768


FULL = Cfg(50000, 800000)


# ----------------------------------------------------------------------------
# Host-side edge preprocessing
# ----------------------------------------------------------------------------

def prep_edges(cfg, src, dst):
    """Bucket edges per core by (dst_tile, src_half); pad to shared sizes."""
    C, T = N_CORES, cfg.t
    counts = np.zeros((C, T, 2), dtype=np.int64)
    per_core = []
    core_of = dst // cfg.nloc
    for c in range(C):
        sel = core_of == c
        es, ed = src[sel].astype(np.int64), dst[sel].astype(np.int64)
        dloc = ed - c * cfg.nloc
        t = dloc // 128
        half = (es >= cfg.split).astype(np.int64)
        order = np.lexsort((es, half, t))
        es, dloc, t, half = es[order], dloc[order], t[order], half[order]
        np.add.at(counts[c], (t, half), 1)
        per_core.append((es, dloc, t, half))

    gmax_th = counts.max(axis=0)                       # (T, 2)
    G = np.maximum(1, -(-gmax_th // 128))              # groups per (t, half)
    base = np.zeros((T, 2), dtype=np.int64)
    acc = 0
    for t in range(T):
        for h in range(2):
            base[t, h] = acc
            acc += G[t, h]
    gtot = acc
    nslot = gtot * 128

    src_idx = np.zeros((C, nslot), dtype=np.int16)
    dst_reb = np.full((C, nslot), -1.0, dtype=np.float16)
    for c in range(C):
        es, dloc, t, half = per_core[c]
        bucket_id = t * 2 + half
        n = len(es)
        starts = np.searchsorted(bucket_id, np.arange(T * 2), side="left")
        pos_in_bucket = np.arange(n) - starts[bucket_id]
        slot = base[t, half] * 128 + pos_in_bucket
        src_idx[c, slot] = (es - np.where(half == 1, cfg.split, 0)).astype(np.int16)
        dst_reb[c, slot] = (dloc - t * 128).astype(np.float32)

    def wrap16(a):
        w = a.reshape(-1, 16).T.copy()                 # (16, nslot/16)
        return np.tile(w, (8, 1))                      # (128, nslot/16)

    src_w = np.stack([wrap16(src_idx[c]) for c in range(C)])
    # dst one-hot source, partition-major per group: dst_w[c][p, g] = dst of
    # slot (p, g)
    dst_w = np.stack([dst_reb[c].reshape(-1, 128).T.copy() for c in range(C)])
    # slot-major rows per tile for the DMA-replicated ST build:
    # dstrow[c][t, g*128 + p] = dst of slot (p, g) of tile t (or -1 pad)
    gt_n = G[:, 0] + G[:, 1]
    gmax = int(gt_n.max())
    dstrow = np.full((C, T, gmax * 128), -1, dtype=np.int8)
    for c in range(C):
        flat = dst_reb[c]
        for t in range(T):
            g0 = int(base[t, 0])
            n_in_tile = int(gt_n[t]) * 128
            dstrow[c, t, :n_in_tile] = flat[
                g0 * 128 : g0 * 128 + n_in_tile
            ].astype(np.int8)

    geom = {"G": G, "base": base, "gtot": gtot, "gmax": gmax}
    return geom, src_w, dst_w, dstrow


def prep_weights(cfg, Ws, als, ars, bs):
    """Combined matmul weights Wc = [W | wl | wr] and packed bias rows."""
    L = cfg.layers
    wc = np.zeros((L, D, HF + 4), dtype=np.float16)
    bp = np.zeros((L, 1, 384), dtype=np.float16)
    for l in range(L):
        W = np.asarray(Ws[l], dtype=np.float32)            # (D, H*F)
        Wh = W.reshape(D, H, D)
        wl = np.einsum("khf,hf->kh", Wh, np.asarray(als[l], np.float32))
        wr = np.einsum("khf,hf->kh", Wh, np.asarray(ars[l], np.float32))
        wc[l, :, :HF] = W.astype(np.float16)
        wc[l, :, HF : HF + 2] = wl.astype(np.float16)
        wc[l, :, HF + 2 : HF + 4] = wr.astype(np.float16)
        b = np.asarray(bs[l], np.float32)
        bp[l, 0, 0:128] = b[0].astype(np.float16)
        bp[l, 0, 128:256] = b[1].astype(np.float16)
        bp[l, 0, 256:384] = (0.5 * (b[0] + b[1])).astype(np.float16)
    return wc, bp


# ----------------------------------------------------------------------------
# Device kernel
# ----------------------------------------------------------------------------

def build(cfg, geom):
    C, T, L = N_CORES, cfg.t, cfg.layers
    G, base, gtot, gmax = geom["G"], geom["base"], geom["gtot"], geom["gmax"]
    nslot = gtot * 128
    NLO = cfg.split

    nc = bacc.Bacc("TRN2", target_bir_lowering=False, debug=False,
                   enable_asserts=False, num_devices=C, num_swdge_queues=2)

    # I/O
    xTb = nc.dram_tensor("xTb", [C, D, cfg.nloc], F16, kind="ExternalInput")
    wc_d = nc.dram_tensor("wc", [L, D, HF + 4], F16, kind="ExternalInput")
    bp_d = nc.dram_tensor("bp", [L, 1, 384], F16, kind="ExternalInput")
    src_d = nc.dram_tensor("srcw", [D, nslot // 16], I16, kind="ExternalInput")
    dst_d = nc.dram_tensor("dstw", [D, gtot], F16, kind="ExternalInput")
    dstrow_d = nc.dram_tensor("dstrow", [T, gmax * 128], I8, kind="ExternalInput")
    oh_d = nc.dram_tensor("onehot", [D, 8], F32, kind="ExternalInput")
    out_d = nc.dram_tensor("out", [cfg.nloc_pad, HF], F32, kind="ExternalOutput")

    # internal DRAM (tbig double-buffered across layers)
    tbig = [nc.dram_tensor(f"tbig{l}", [cfg.n, ROW16], F16) for l in range(L)]
    THALF = (T + 1) // 2
    ch0 = min(THALF * 128, cfg.nloc)
    ch1 = cfg.nloc - ch0
    hT_own0 = nc.dram_tensor("hT_own0", [D, ch0], F16)
    hT_own1 = nc.dram_tensor("hT_own1", [D, ch1], F16)
    hT_all0 = nc.dram_tensor("hT_all0", [C, D, ch0], F16)
    hT_all1 = nc.dram_tensor("hT_all1", [C, D, ch1], F16)

    def chunk_list(lo, hi):
        out = []
        t = lo
        while t < hi:
            nt = 2 if (t + 2 <= hi and t + 1 != T - 1) else 1
            out.append((t, nt))
            t += nt
        return out

    with tile.TileContext(nc) as tc:
        with (
            tc.tile_pool(name="const", bufs=1) as cpool,
            tc.tile_pool(name="work", bufs=2) as pool,
            tc.tile_pool(name="p1w", bufs=4) as p1w,
            tc.tile_pool(name="gtp", bufs=3) as gtp,
            tc.tile_pool(name="ppA", bufs=3, space="PSUM") as ppA,
            tc.tile_pool(name="ppB", bufs=1, space="PSUM") as ppB,
        ):
            # ---- constants ----
            src_sb = cpool.tile([D, nslot // 16], I16, tag="src_sb")
            nc.sync.dma_start(out=src_sb[:], in_=src_d[:])
            dst_sb = cpool.tile([D, gtot], F16, tag="dst_sb")
            nc.sync.dma_start(out=dst_sb[:], in_=dst_d[:])
            oh_sb = cpool.tile([D, 8], F32, tag="oh_sb")
            nc.sync.dma_start(out=oh_sb[:], in_=oh_d[:])
            wc_sb = cpool.tile([D, L * (HF + 4)], F16, tag="wc_sb")
            bp_sb = cpool.tile([1, L * 384], F16, tag="bp_sb")
            for l in range(L):
                nc.sync.dma_start(
                    out=wc_sb[:, l * (HF + 4) : (l + 1) * (HF + 4)], in_=wc_d[l]
                )
                nc.sync.dma_start(
                    out=bp_sb[:, l * 384 : (l + 1) * 384], in_=bp_d[l]
                )

            it16 = cpool.tile([D, D], I16, tag="it16")
            nc.gpsimd.iota(it16[:], pattern=[[1, D]], base=0, channel_multiplier=0)
            iota_f = cpool.tile([D, D], F16, tag="iota_f")
            nc.vector.tensor_copy(iota_f[:], it16[:])

            ip32 = cpool.tile([D, 1], I32, tag="ip32")
            nc.gpsimd.iota(ip32[:], pattern=[[1, 1]], base=0, channel_multiplier=1)
            iota_p8 = cpool.tile([D, 1], I8, tag="iota_p8")
            nc.vector.tensor_copy(iota_p8[:], ip32[:])

            ones_row = cpool.tile([1, D], F16, tag="ones_row")
            nc.vector.memset(ones_row[:], 1.0)
            ident = cpool.tile([D, D], F16, tag="ident")
            make_identity(nc, ident[:])

            epsc = cpool.tile([D, 1], F32, tag="epsc")
            nc.vector.memset(epsc[:], 1e-30)
            halfc = cpool.tile([D, 1], F32, tag="halfc")
            nc.vector.memset(halfc[:], 0.5)
            slopec = cpool.tile([D, 1], F32, tag="slopec")
            nc.vector.memset(slopec[:], NEG_SLOPE)
            zc32 = cpool.tile([D, 1], F32, tag="zc32")
            nc.vector.memset(zc32[:], 0.0)
            zc16 = cpool.tile([D, 1], F16, tag="zc16")
            nc.vector.memset(zc16[:], 0.0)
            onesc = cpool.tile([D, 1], F16, tag="onesc")
            nc.vector.memset(onesc[:], 1.0)

            # layer-indexed staging (layers overlap: tbig double-buffered)
            er_stage = cpool.tile([D, L, T, 2, 8], F32, tag="er_stage")
            hT_stage = cpool.tile([D, cfg.nloc_pad], F16, tag="hT_stage")
            brep_t = cpool.tile([D, L, 384], F32, tag="brep")

            def emit_layer_prep(l):
                brep = brep_t[:, l]
                for k in range(3):
                    bps = ppA.tile([D, HF + 4], F32, tag="p1")
                    nc.tensor.matmul(
                        bps[:, 0:128], lhsT=ones_row[:],
                        rhs=bp_sb[:, l * 384 + k * 128 : l * 384 + (k + 1) * 128],
                        start=True, stop=True,
                    )
                    nc.vector.tensor_tensor(
                        out=brep[:, k * 128 : (k + 1) * 128], in0=bps[:, 0:128],
                        in1=zc32[:, 0:1].to_broadcast([D, D]),
                        op=mybir.AluOpType.add,
                    )
                nc.vector.memset(er_stage[:, l], 0.0)

            def emit_p1_chunk(l, cb, t, nt):
                wcl = wc_sb[:, l * (HF + 4) : l * (HF + 4) + HF + 4]
                tb = tbig[l]
                ws = [cfg.w_last if t + j == T - 1 else 128 for j in range(nt)]
                w2 = sum(ws)
                xt = p1w.tile([D, 256], F16, tag="xt")
                if l == 0:
                    src_ap = xTb[cb, :, t * 128 : t * 128 + w2]
                elif t < THALF:
                    src_ap = hT_all0[cb, :, t * 128 : t * 128 + w2]
                else:
                    c0 = t * 128 - ch0
                    src_ap = hT_all1[cb, :, c0 : c0 + w2]
                nc.sync.dma_start(out=xt[:, :w2], in_=src_ap)
                stage = p1w.tile([D, 2, 384], F16, tag="stage")
                for j in range(nt):
                    wj = ws[j]
                    ps1 = ppA.tile([D, HF + 4], F32, tag="p1")
                    nc.tensor.matmul(
                        ps1[:wj, :], lhsT=xt[:, j * 128 : j * 128 + wj],
                        rhs=wcl, start=True, stop=True,
                    )
                    # f0 | f1 at cols 0:128 / 129:257 (stride 129)
                    sv = stage[:, j, 0:258].rearrange("p (h v) -> p h v", h=2)
                    nc.scalar.activation(
                        sv[:wj, :, 0:128],
                        ps1[:wj, 0:HF].rearrange("p (h v) -> p h v", h=2),
                        mybir.ActivationFunctionType.Copy,
                    )
                    nc.scalar.activation(
                        stage[:wj, j, 258:262].bitcast(F32),
                        ps1[:wj, HF : HF + 2],
                        mybir.ActivationFunctionType.Copy,
                    )
                    nc.vector.tensor_tensor(
                        out=er_stage[:wj, l, t + j, :, cb],
                        in0=ps1[:wj, HF + 2 : HF + 4],
                        in1=zc32[:wj, 0:1].to_broadcast([wj, 2]),
                        op=mybir.AluOpType.add,
                    )
                for col in (128, 257):
                    nc.vector.tensor_tensor(
                        out=stage[:, :, col : col + 1],
                        in0=onesc[:, 0:1].unsqueeze(1).to_broadcast([D, 2, 1]),
                        in1=zc16[:, 0:1].unsqueeze(1).to_broadcast([D, 2, 1]),
                        op=mybir.AluOpType.add,
                    )
                n0 = cb * cfg.nloc + t * 128
                if nt == 2:
                    tbv = tb[n0 : n0 + 256, :].rearrange("(j p) v -> p j v", p=128)
                    nc.sync.dma_start(out=tbv[:, :, 0:262], in_=stage[:, :, 0:262])
                else:
                    nc.sync.dma_start(
                        out=tb[n0 : n0 + ws[0], 0:262], in_=stage[: ws[0], 0, 0:262]
                    )

            def emit_p2_tile(l, t):
                brep = brep_t[:, l]
                tb = tbig[l]
                g_lo, g_hi = int(G[t, 0]), int(G[t, 1])
                gt_n = g_lo + g_hi
                goff = int(base[t, 0])

                gt = gtp.tile([D, gmax, ROW16], F16, tag="gt")
                nc.gpsimd.dma_gather(
                    out_ap=gt[:, 0:g_lo, :],
                    in_ap=tb[0:NLO, :],
                    idxs_ap=src_sb[:, goff * 8 : (goff + g_lo) * 8],
                    num_idxs=g_lo * 128,
                    num_idxs_reg=g_lo * 128,
                    elem_size=ROW16,
                    queue_num=0,
                    single_packet=False,
                )
                nc.gpsimd.dma_gather(
                    out_ap=gt[:, g_lo:gt_n, :],
                    in_ap=tb[NLO : cfg.n, :],
                    idxs_ap=src_sb[:, (goff + g_lo) * 8 : (goff + gt_n) * 8],
                    num_idxs=g_hi * 128,
                    num_idxs_reg=g_hi * 128,
                    elem_size=ROW16,
                    queue_num=1,
                    single_packet=False,
                )

                # er2[d, h] = own-core er of node t*128+d
                tmp8 = pool.tile([D, 2, 8], F32, tag="tmp8")
                nc.vector.tensor_tensor(
                    out=tmp8[:],
                    in0=er_stage[:, l, t],
                    in1=oh_sb[:].unsqueeze(1).to_broadcast([D, 2, 8]),
                    op=mybir.AluOpType.mult,
                )
                er2f = pool.tile([D, 2], F32, tag="er2f")
                nc.vector.reduce_sum(er2f[:], tmp8[:], axis=mybir.AxisListType.X)
                er2 = pool.tile([D, 2], F16, tag="er2")
                nc.vector.tensor_tensor(
                    out=er2[:], in0=er2f[:],
                    in1=zc32[:, 0:1].to_broadcast([D, 2]),
                    op=mybir.AluOpType.add,
                )

                # transposed one-hot from DMA-replicated dst row; ere = ST^T@er2
                drep = pool.tile([D, gmax * 128], I8, tag="drep")
                nc.sync.dma_start(
                    out=drep[:, 0 : gt_n * 128],
                    in_=dstrow_d[t : t + 1, 0 : gt_n * 128].to_broadcast(
                        [D, gt_n * 128]
                    ),
                )
                ST = pool.tile([D, gmax, D], F16, tag="ST")
                nc.vector.tensor_tensor(
                    out=ST[:, 0:gt_n],
                    in0=drep[:, 0 : gt_n * 128].rearrange("p (g e) -> p g e", e=D),
                    in1=iota_p8[:, 0:1].unsqueeze(1).to_broadcast([D, gt_n, D]),
                    op=mybir.AluOpType.is_equal,
                )
                ps_ere = ppB.tile([D, gmax, 2], F32, tag="ere")
                for g in range(gt_n):
                    nc.tensor.matmul(
                        ps_ere[:, g, :],
                        lhsT=ST[:, g, :],
                        rhs=er2[:],
                        start=True, stop=True,
                    )

                # u = el_src + er_dst ; w = exp(max(u, 0.2u))
                elv = gt[:, 0:gt_n, 258:262].bitcast(F32)
                u = pool.tile([D, gmax, 2], F32, tag="u")
                nc.vector.tensor_tensor(
                    out=u[:, 0:gt_n], in0=ps_ere[:, 0:gt_n], in1=elv,
                    op=mybir.AluOpType.add,
                )
                lr = pool.tile([D, gmax, 2], F32, tag="lr")
                nc.vector.tensor_tensor(
                    out=lr[:, 0:gt_n], in0=u[:, 0:gt_n],
                    in1=slopec[:, 0:1].unsqueeze(1).to_broadcast([D, gt_n, 2]),
                    op=mybir.AluOpType.mult,
                )
                nc.vector.tensor_tensor(
                    out=u[:, 0:gt_n], in0=u[:, 0:gt_n], in1=lr[:, 0:gt_n],
                    op=mybir.AluOpType.max,
                )
                w16 = pool.tile([D, gmax, 2], F16, tag="w16")
                nc.scalar.activation(
                    w16[:, 0:gt_n], u[:, 0:gt_n],
                    mybir.ActivationFunctionType.Exp,
                )

                # one-hot S[e, d] and per-head scaled copies
                S = pool.tile([D, gmax, D], F16, tag="S")
                nc.vector.tensor_tensor(
                    out=S[:, 0:gt_n],
                    in0=dst_sb[:, goff : goff + gt_n].unsqueeze(2).to_broadcast(
                        [D, gt_n, D]
                    ),
                    in1=iota_f[:].unsqueeze(1).to_broadcast([D, gt_n, D]),
                    op=mybir.AluOpType.is_equal,
                )
                Sw0 = pool.tile([D, gmax, D], F16, tag="Sw0")
                nc.vector.tensor_tensor(
                    out=Sw0[:, 0:gt_n],
                    in0=S[:, 0:gt_n],
                    in1=w16[:, 0:gt_n, 0:1].to_broadcast([D, gt_n, D]),
                    op=mybir.AluOpType.mult,
                )
                Sw1 = pool.tile([D, gmax, D], F16, tag="Sw1")
                nc.vector.tensor_tensor(
                    out=Sw1[:, 0:gt_n],
                    in0=S[:, 0:gt_n],
                    in1=w16[:, 0:gt_n, 1:2].to_broadcast([D, gt_n, D]),
                    op=mybir.AluOpType.mult,
                )

                # aggregation: two sequential chains sharing one bank
                # (start=True clears accumulate flags bank-wide, so the
                # chains must not interleave)
                ps_agg = ppA.tile([D, 258], F32, tag="agg")
                ps_a0 = ps_agg[:, 0:129]
                ps_a1 = ps_agg[:, 129:258]
                for g in range(gt_n):
                    nc.tensor.matmul(
                        ps_a0,
                        lhsT=Sw0[:, g, :],
                        rhs=gt[:, g, 0:129],
                        start=(g == 0),
                        stop=(g == gt_n - 1),
                    )
                for g in range(gt_n):
                    nc.tensor.matmul(
                        ps_a1,
                        lhsT=Sw1[:, g, :],
                        rhs=gt[:, g, 129:258],
                        start=(g == 0),
                        stop=(g == gt_n - 1),
                    )

                # ---- finalize ----
                rsb = pool.tile([D, 2], F32, tag="rsb")
                nc.vector.tensor_tensor(
                    out=rsb[:, 0:1], in0=ps_agg[:, 128:129], in1=epsc[:],
                    op=mybir.AluOpType.max,
                )
                nc.vector.tensor_tensor(
                    out=rsb[:, 1:2], in0=ps_agg[:, 257:258], in1=epsc[:],
                    op=mybir.AluOpType.max,
                )
                nc.vector.reciprocal(rsb[:], rsb[:])
                if l == 0:
                    rh = pool.tile([D, 2], F32, tag="rh")
                    nc.vector.tensor_tensor(
                        out=rh[:], in0=rsb[:],
                        in1=halfc[:, 0:1].to_broadcast([D, 2]),
                        op=mybir.AluOpType.mult,
                    )
                    t0 = pool.tile([D, D], F32, tag="t0")
                    nc.vector.tensor_tensor(
                        out=t0[:], in0=ps_agg[:, 0:128],
                        in1=rh[:, 0:1].to_broadcast([D, D]),
                        op=mybir.AluOpType.mult,
                    )
                    t1 = pool.tile([D, D], F32, tag="t1")
                    nc.vector.tensor_tensor(
                        out=t1[:], in0=ps_agg[:, 129:257],
                        in1=rh[:, 1:2].to_broadcast([D, D]),
                        op=mybir.AluOpType.mult,
                    )
                    nc.vector.tensor_tensor(
                        out=t0[:], in0=t0[:], in1=t1[:], op=mybir.AluOpType.add
                    )
                    ht16 = pool.tile([D, D], F16, tag="ht16")
                    nc.vector.tensor_tensor(
                        out=ht16[:], in0=t0[:], in1=brep[:, 256:384],
                        op=mybir.AluOpType.add,
                    )
                    pst = ppB.tile([D, D], F16, tag="pt")
                    nc.tensor.transpose(pst[:], ht16[:], ident[:])
                    nc.vector.tensor_tensor(
                        out=hT_stage[:, t * 128 : (t + 1) * 128],
                        in0=pst[:],
                        in1=zc16[:, 0:1].to_broadcast([D, D]),
                        op=mybir.AluOpType.add,
                    )
                    if t == THALF - 1:
                        nc.sync.dma_start(out=hT_own0[:], in_=hT_stage[:, 0:ch0])
                        with nc.named_scope("cc0"):
                            nc.gpsimd.collective_compute(
                                "AllGather",
                                mybir.AluOpType.bypass,
                                replica_groups=[list(range(C))],
                                ins=[hT_own0[:]],
                                outs=[hT_all0[:]],
                            )
                else:
                    osb = pool.tile([D, HF], F32, tag="osb")
                    nc.vector.tensor_tensor(
                        out=osb[:, 0:128], in0=ps_agg[:, 0:128],
                        in1=rsb[:, 0:1].to_broadcast([D, D]),
                        op=mybir.AluOpType.mult,
                    )
                    nc.vector.tensor_tensor(
                        out=osb[:, 128:256], in0=ps_agg[:, 129:257],
                        in1=rsb[:, 1:2].to_broadcast([D, D]),
                        op=mybir.AluOpType.mult,
                    )
                    nc.vector.tensor_tensor(
                        out=osb[:], in0=osb[:], in1=brep[:, 0:256],
                        op=mybir.AluOpType.add,
                    )
                    nc.scalar.dma_start(
                        out=out_d[t * 128 : (t + 1) * 128, :], in_=osb[:]
                    )

            # ================= emission schedule =================
            emit_layer_prep(0)
            with nc.named_scope("p1_l0"):
                for cb in range(C):
                    for (t, nt) in chunk_list(0, T):
                        emit_p1_chunk(0, cb, t, nt)
            emit_layer_prep(1)

            # layer-1 P1 chunks for the first hT half, interleaved into the
            # second half of layer-0's edge loop (after cc0 fires)
            l1h0 = [(cb, t, nt) for cb in range(C) for (t, nt) in chunk_list(0, THALF)]
            idx = 0
            with nc.named_scope("p2_l0"):
                for t in range(T):
                    emit_p2_tile(0, t)
                    if t >= THALF:
                        quota = -(-(len(l1h0) - idx) // (T - t))
                        for _ in range(quota):
                            if idx < len(l1h0):
                                cb_, t_, nt_ = l1h0[idx]
                                with nc.named_scope("p1_l1"):
                                    emit_p1_chunk(1, cb_, t_, nt_)
                                idx += 1
            while idx < len(l1h0):
                cb_, t_, nt_ = l1h0[idx]
                with nc.named_scope("p1_l1"):
                    emit_p1_chunk(1, cb_, t_, nt_)
                idx += 1

            nc.sync.dma_start(out=hT_own1[:], in_=hT_stage[:, ch0 : cfg.nloc])
            with nc.named_scope("cc"):
                nc.gpsimd.collective_compute(
                    "AllGather",
                    mybir.AluOpType.bypass,
                    replica_groups=[list(range(C))],
                    ins=[hT_own1[:]],
                    outs=[hT_all1[:]],
                )
            with nc.named_scope("p1_l1"):
                for cb in range(C):
                    for (t, nt) in chunk_list(THALF, T):
                        emit_p1_chunk(1, cb, t, nt)
            with nc.named_scope("p2_l1"):
                for t in range(T):
                    emit_p2_tile(1, t)
    nc.compile()
    return nc


# ----------------------------------------------------------------------------
# Entry point
# ----------------------------------------------------------------------------

def run_gat(cfg, x, Ws, als, ars, bs, src, dst, trace=False):
    geom, src_w, dst_w, dstrow = prep_edges(cfg, np.asarray(src), np.asarray(dst))
    wc, bp = prep_weights(cfg, Ws, als, ars, bs)

    x = np.asarray(x, dtype=np.float32)
    xTb = np.ascontiguousarray(
        x.reshape(N_CORES, cfg.nloc, D).transpose(0, 2, 1)
    ).astype(np.float16)

    onehots = []
    for c in range(N_CORES):
        oh = np.zeros((D, 8), dtype=np.float32)
        oh[:, c] = 1.0
        onehots.append(oh)

    nc = build(cfg, geom)
    in_maps = []
    for c in range(N_CORES):
        in_maps.append({
            "xTb": xTb,
            "wc": wc,
            "bp": bp,
            "srcw": src_w[c],
            "dstw": dst_w[c].astype(np.float16),
            "dstrow": dstrow[c],
            "onehot": onehots[c],
        })
    res = run_bass_kernel_spmd(nc, in_maps, list(range(N_CORES)), trace=trace)
    outs = [res.results[c]["out"][: cfg.nloc] for c in range(N_CORES)]
    out = np.concatenate(outs, axis=0).reshape(cfg.n, H, D)
    return out, res


def kernel(x, Ws, als, ars, bs, src, dst):
    out, _ = run_gat(FULL, x, Ws, als, ars, bs, src, dst, trace=False)
    return out.astype(np.float32)
